# revision 33
# baseline (speedup 1.0000x reference)
"""DiT block kernel for 8 Trainium2 NeuronCores (Bass/Tile).

Sharding: each core owns a 256-wide query slice of the sequence (all batches,
all heads).
 - LN1/modulate/QKV/rmsnorm computed on own rows; K^T (bf16) and V (fp8,
   pre-interleaved with a ones-column per head) are gathered with ONE fused
   AllGather so every core holds full K/V.
 - Attention bias is pre-transposed on host to [H, m, n] and sliced per core
   along n, so every bias element is read exactly once across the machine.
   Bias is pre-loaded into PSUM via an identity matmul, the scores matmul
   accumulates on top, ScalarE applies exp (PSUM -> SBUF fp8e5m2).
 - o^T accumulated on PE in fp8 DoubleRow over m-tile pairs, with the ones
   column so the softmax denominator rides along as psum row 64; the divide
   is folded into the o^T evacuation via a tiny broadcast matmul.
 - proj/MLP are row-local; outputs concatenated on host.

Engine balance: per-head rmsnorm scaling runs on the Pool engine; LN1 stats
overlap the adaLN matmuls; proj/w2 weights prefetch under attention/MLP.
"""

import contextlib

import numpy as np
import ml_dtypes

import concourse.bacc as bacc
import concourse.tile as tile
from concourse import mybir
from concourse.bass_utils import run_bass_kernel_spmd

bf16 = ml_dtypes.bfloat16
F32 = mybir.dt.float32
BF16 = mybir.dt.bfloat16
F8 = mybir.dt.float8e4
F8E5 = mybir.dt.float8e5
AF = mybir.ActivationFunctionType
AL = mybir.AluOpType

B, N, C = 4, 2048, 768
H, D = 12, 64
FFN = 2048
NCORE = 8
NS = N // NCORE          # 256 queries per core
R = B * NS               # 1024 rows per core
RT = R // 128            # 8 row tiles
KT = C // 128            # 6 contraction tiles over C
FT = FFN // 128          # 16 FFN row tiles
EPS_LN, EPS_RMS = 1e-6, 1e-8

VW = H * (D + 1)         # 780: v row width with per-head ones column
VP = 784                 # padded v row stride (fp8 DoubleRow ldweights needs
                         # 16B-aligned tile strides)
KV_K = C * R             # elems (= bytes, fp8) of k^T shard block
KV_TOT = KV_K + R * VP   # fused shard bytes: k^T | v'


def _bc(ap, parts=128):
    """partition-stride-0 broadcast AP (DRAM source)."""
    import dataclasses
    return dataclasses.replace(ap, ap=[[0, parts]] + list(ap.ap))


def _ln_mod(nc, pool, src_ap, sc_bc, sh_bc, dst_bf, eps_tile):
    """dst = LN(src) * sc + sh   (sc already includes the +1)."""
    stats = pool.tile([128, 2, 6], F32, tag="ln_stats", name="ln_stats")
    nc.vector.bn_stats(out=stats[:, 0, :], in_=src_ap[:, 0:384])
    nc.vector.bn_stats(out=stats[:, 1, :], in_=src_ap[:, 384:768])
    mv = pool.tile([128, 2], F32, tag="ln_mv", name="ln_mv")
    nc.vector.bn_aggr(out=mv, in_=stats)
    rstd = pool.tile([128, 1], F32, tag="ln_rstd", name="ln_rstd")
    nc.scalar.activation(out=rstd, in_=mv[:, 1:2], func=AF.Sqrt, bias=eps_tile)
    nc.vector.reciprocal(out=rstd, in_=rstd)
    t1 = pool.tile([128, C], F32, tag="ln_t1", name="ln_t1")
    nc.vector.tensor_scalar(out=t1, in0=src_ap, scalar1=mv[:, 0:1], scalar2=rstd,
                            op0=AL.subtract, op1=AL.mult)
    nc.vector.tensor_tensor(out=t1, in0=t1, in1=sc_bc, op=AL.mult)
    nc.vector.tensor_tensor(out=dst_bf, in0=t1, in1=sh_bc, op=AL.add)


def build(collective=True, repeat=1, ablate=()):
    ablate = frozenset(ablate)
    nc = bacc.Bacc("TRN2", target_bir_lowering=False, debug=False,
                   num_devices=NCORE)

    x_in = nc.dram_tensor("x", [R, C], F32, kind="ExternalInput")
    cT_in = nc.dram_tensor("cT", [C, B], F32, kind="ExternalInput")
    bias_in = nc.dram_tensor("bias_t", [H, N, NS], BF16, kind="ExternalInput")
    adw_in = nc.dram_tensor("adaln_wT", [C, 6 * C], BF16, kind="ExternalInput")
    adb_in = nc.dram_tensor("adaln_b4", [B, 6 * C], F32, kind="ExternalInput")
    qkvw_in = nc.dram_tensor("qkv_wT", [C, 3 * C], BF16, kind="ExternalInput")
    qkvb_in = nc.dram_tensor("qkv_b_bc", [128, 3 * C], F32, kind="ExternalInput")
    qsc_in = nc.dram_tensor("qscale_bc", [128, C], BF16, kind="ExternalInput")
    ksc_in = nc.dram_tensor("kscale_bc", [128, C], BF16, kind="ExternalInput")
    pw_in = nc.dram_tensor("proj_wT", [C, C], BF16, kind="ExternalInput")
    pb_in = nc.dram_tensor("proj_b_bc", [128, C], F32, kind="ExternalInput")
    w1_in = nc.dram_tensor("w1T", [C, FFN], BF16, kind="ExternalInput")
    w3_in = nc.dram_tensor("w3T", [C, FFN], BF16, kind="ExternalInput")
    w2_in = nc.dram_tensor("w2T", [FFN, C], BF16, kind="ExternalInput")
    w2b_in = nc.dram_tensor("w2_b_bc", [128, C], F32, kind="ExternalInput")
    id_in = nc.dram_tensor("id128", [128, 128], BF16, kind="ExternalInput")
    out_t = nc.dram_tensor("out", [R, C], F32, kind="ExternalOutput")

    with tile.TileContext(nc, num_cores=NCORE) as tc, contextlib.ExitStack() as top:
        consts = top.enter_context(tc.tile_pool(name="consts", bufs=1))
        dram = top.enter_context(tc.tile_pool(name="dram", bufs=1, space="DRAM"))
        keep = top.enter_context(tc.tile_pool(name="keep", bufs=1))

        eps_ln = consts.tile([128, 1], F32)
        nc.vector.memset(eps_ln, EPS_LN)
        id_sb = consts.tile([128, 128], BF16)
        nc.sync.dma_start(out=id_sb, in_=id_in[:, :])
        ones_sb = consts.tile([128, 128], BF16)
        nc.vector.memset(ones_sb, 1.0)

        for _rep in range(repeat):
            with contextlib.ExitStack() as ctx:
                qT_sb = keep.tile([128, KT, R], F8)       # packed q^T (fp8)
                oT_sb = keep.tile([128, KT, R], BF16)     # packed normalized o^T
                h2T_sb = keep.tile([128, KT, R], BF16)    # packed LN2-mod x2^T
                mod_dram = dram.tile([B, 6 * C], F32)
                x1_sb = keep.tile([128, RT, C], F32)      # resident input rows
                x2_sb = keep.tile([128, RT, C], F32)      # post-attn residual
                st_sb = keep.tile([128, RT, 2, 6], F32)   # LN1 bn stats
                mv_sb = keep.tile([128, RT, 2], F32)      # LN1 mean/var
                rs_sb = keep.tile([128, RT, 1], F32)      # LN1 rstd

                # ===== P0: adaLN modulation, overlapped with LN1 stats ======
                with tc.tile_pool(name="p0", bufs=1) as p0, \
                     tc.tile_pool(name="p0c", bufs=3) as p0c, \
                     tc.tile_pool(name="p0ps", bufs=2, space="PSUM") as p0ps:
                    cT_sb = p0.tile([128, KT, B], F32)
                    nc.sync.dma_start(
                        out=cT_sb, in_=cT_in.rearrange("(t p) b -> p t b", p=128))
                    scT = p0.tile([128, KT, B], BF16)
                    nc.scalar.activation(out=scT, in_=cT_sb, func=AF.Silu)

                    # LN1 stats for all row tiles: x DMA on gpsimd queue, stats
                    # on DVE — both run under the adaLN matmuls below.
                    for rt in range(RT):
                        nc.gpsimd.dma_start(
                            out=x1_sb[:, rt, :],
                            in_=x_in[rt * 128:(rt + 1) * 128, :])
                        nc.vector.bn_stats(out=st_sb[:, rt, 0, :],
                                           in_=x1_sb[:, rt, 0:384])
                        nc.vector.bn_stats(out=st_sb[:, rt, 1, :],
                                           in_=x1_sb[:, rt, 384:768])
                        nc.vector.bn_aggr(out=mv_sb[:, rt], in_=st_sb[:, rt])
                        nc.scalar.activation(out=rs_sb[:, rt], in_=mv_sb[:, rt, 1:2],
                                             func=AF.Sqrt, bias=eps_ln)
                        nc.vector.reciprocal(out=rs_sb[:, rt], in_=rs_sb[:, rt])

                    adwg = adw_in.rearrange("(t p) j -> p t j", p=128)
                    adb_sb = p0.tile([B, 6 * C], F32)
                    nc.sync.dma_start(out=adb_sb, in_=adb_in[:, :])
                    mod_sb = p0.tile([B, 6 * C], F32)
                    for big in range(3):
                        bsl = slice(big * 1536, (big + 1) * 1536)
                        adw_t = p0c.tile([128, KT, 1536], BF16, tag="adw_t",
                                         name="adw_t")
                        nc.sync.dma_start(out=adw_t, in_=adwg[:, :, bsl])
                        for sub in range(3):
                            c0 = big * 1536 + sub * 512
                            sl = slice(c0, c0 + 512)
                            psM = p0ps.tile([B, 512], F32, tag="psM", name="psM")
                            for kt in range(KT):
                                nc.tensor.matmul(psM, lhsT=scT[:, kt, :],
                                                 rhs=adw_t[:, kt,
                                                           sub * 512:(sub + 1) * 512],
                                                 start=(kt == 0), stop=(kt == KT - 1))
                            nc.vector.tensor_tensor(out=mod_sb[:, sl], in0=psM,
                                                    in1=adb_sb[:, sl], op=AL.add)
                        nc.gpsimd.dma_start(out=mod_dram[:, bsl], in_=mod_sb[:, bsl])

                # ===== P1+P2: LN1 modulate, QKV (k,v first), rmsnorm =====
                # qkv weight columns are host-permuted to [k | v | q] so the K/V
                # side finishes first and the AllGather overlaps Q-side compute.
                # two half-shards (batches 0-1 | 2-3): the first AllGather
                # fires mid-way through the k/v loop and overlaps the rest of
                # P2; attention on b0/b1 overlaps the second gather.
                RH = R // 2
                KV_H = KT * 128 * RH
                TOT_H = KV_H + RH * VP
                addr = "Shared" if collective else "Local"
                kv_shard = [dram.tile([1, TOT_H], F8, tag=f"kvs{h}",
                                      name=f"kvs{h}") for h in range(2)]
                kv_all = [dram.tile([NCORE, TOT_H], F8, addr_space=addr,
                                    tag=f"kva{h}", name=f"kva{h}")
                          for h in range(2)]
                kv_k_view = [s[0, 0:KV_H].rearrange("(t p n) -> p t n",
                                                    p=128, t=KT)
                             for s in kv_shard]
                kv_v_view = [s[0, KV_H:].rearrange("(r c) -> r c", c=VP)
                             for s in kv_shard]

                def emit_gather(h):
                    if "ag" in ablate:
                        return
                    if collective:
                        nc.gpsimd.collective_compute(
                            "AllGather", AL.bypass,
                            replica_groups=[list(range(NCORE))],
                            ins=[kv_shard[h].opt()], outs=[kv_all[h].opt()],
                        )
                    else:
                        for cc in range(2):
                            nc.scalar.dma_start(out=kv_all[h][cc:cc + 1, :],
                                                in_=kv_shard[h][:, :])
                with tc.tile_pool(name="bc1", bufs=1) as bc1, \
                     tc.tile_pool(name="p2", bufs=1) as p2, \
                     tc.tile_pool(name="p2w", bufs=2) as p2w, \
                     tc.tile_pool(name="p2ps", bufs=4, space="PSUM") as p2ps:
                    msa_sc, msa_sh = [], []
                    for b in range(B):
                        # host folded the +1 into adaln_b4's scale segments
                        sc = bc1.tile([128, C], F32, tag=f"sc1_{b}", name=f"sc1_{b}")
                        nc.sync.dma_start(out=sc, in_=_bc(mod_dram[b, C:2 * C]))
                        sh = bc1.tile([128, C], F32, tag=f"sh1_{b}", name=f"sh1_{b}")
                        nc.sync.dma_start(out=sh, in_=_bc(mod_dram[b, 0:C]))
                        msa_sc.append(sc)
                        msa_sh.append(sh)

                    qkvw_sb = p2.tile([128, KT, 3 * C], BF16)
                    nc.scalar.dma_start(
                        out=qkvw_sb, in_=qkvw_in.rearrange("(t p) j -> p t j", p=128))
                    qkvb_sb = p2.tile([128, 3 * C], F32)
                    nc.scalar.dma_start(out=qkvb_sb, in_=qkvb_in[:, :])
                    qsc_sb = p2.tile([128, C], BF16)
                    nc.scalar.dma_start(out=qsc_sb, in_=qsc_in[:, :])
                    ksc_sb = p2.tile([128, C], BF16)
                    nc.scalar.dma_start(out=ksc_sb, in_=ksc_in[:, :])

                    h1T_sb = p2.tile([128, KT, R], BF16)
                    for rt in range(RT):
                        t1 = p2w.tile([128, C], F32, tag="m1_t1", name="m1_t1")
                        nc.vector.tensor_scalar(
                            out=t1, in0=x1_sb[:, rt, :], scalar1=mv_sb[:, rt, 0:1],
                            scalar2=rs_sb[:, rt], op0=AL.subtract, op1=AL.mult)
                        nc.vector.tensor_tensor(out=t1, in0=t1,
                                                in1=msa_sc[rt // 2], op=AL.mult)
                        h1_t = p2w.tile([128, C], BF16, tag="h1_t", name="h1_t")
                        nc.vector.tensor_tensor(out=h1_t, in0=t1,
                                                in1=msa_sh[rt // 2], op=AL.add)
                        nc.sync.dma_start_transpose(
                            out=h1T_sb[:, :, rt * 128:(rt + 1) * 128], in_=h1_t)

                    def qkv_mm(rt, c0, cw):
                        psQ = p2ps.tile([128, 512], F32, tag="psQ", name="psQ")
                        for kt in range(KT):
                            nc.tensor.matmul(
                                psQ[:, 0:cw],
                                lhsT=h1T_sb[:, kt, rt * 128:(rt + 1) * 128],
                                rhs=qkvw_sb[:, kt, c0:c0 + cw],
                                start=(kt == 0), stop=(kt == KT - 1))
                        return psQ

                    def pool_headmul(dst, ss):
                        for h in range(H):
                            hs = slice(h * D, (h + 1) * D)
                            nc.gpsimd.tensor_scalar(
                                out=dst[:, hs], in0=dst[:, hs],
                                scalar1=ss[:, h:h + 1], scalar2=None, op0=AL.mult)

                    def rms_apply(t, scale_sb, dst, tagp):
                        """t: [128, 768] bf16 -> dst normalized bf16."""
                        sq = p2w.tile([128, C], BF16, tag=f"sq{tagp}", name="sq")
                        nc.vector.tensor_tensor(out=sq, in0=t, in1=t, op=AL.mult)
                        ss = p2w.tile([128, H], F32, tag=f"ss{tagp}", name="ss")
                        nc.vector.tensor_reduce(
                            out=ss, in_=sq.rearrange("p (h d) -> p h d", d=D),
                            axis=mybir.AxisListType.X, op=AL.add)
                        nc.scalar.activation(out=ss, in_=ss, func=AF.Sqrt,
                                             scale=1.0 / D)
                        nc.vector.tensor_scalar_add(out=ss, in0=ss, scalar1=EPS_RMS)
                        nc.vector.reciprocal(out=ss, in_=ss)
                        nc.vector.tensor_tensor(out=dst, in0=t, in1=scale_sb,
                                                op=AL.mult)
                        pool_headmul(dst, ss)

                    # ---- k,v side ----
                    for rt in range(RT):
                        rsl = slice(rt * 128, (rt + 1) * 128)
                        kv_t = p2w.tile([128, C], BF16, tag="kv_t", name="kv_t")
                        v8_t = p2w.tile([128, VP], F8, tag="v8_t", name="v8_t")
                        v8h = v8_t[:, 0:VW].rearrange("p (h e) -> p h e", e=D + 1)
                        nc.vector.memset(v8h[:, :, D:D + 1], 1.0)
                        nc.vector.memset(v8_t[:, VW:VP], 0.0)
                        psK = qkv_mm(rt, 0, 512)
                        nc.vector.tensor_tensor(
                            out=kv_t[:, 0:512], in0=psK,
                            in1=qkvb_sb[:, 0:512], op=AL.add)
                        psV = qkv_mm(rt, 1024, 512)
                        with nc.allow_low_precision(reason="fp8 v path validated"):
                            nc.vector.tensor_tensor(
                                out=v8h[:, 4:12, 0:D],
                                in0=psV.rearrange("p (h d) -> p h d", d=D),
                                in1=qkvb_sb[:, 1024:1536].rearrange(
                                    "p (h d) -> p h d", d=D),
                                op=AL.add)
                        # middle chunk straddles k|v: split the evacuation
                        psM2 = qkv_mm(rt, 512, 512)
                        nc.vector.tensor_tensor(
                            out=kv_t[:, 512:768], in0=psM2[:, 0:256],
                            in1=qkvb_sb[:, 512:768], op=AL.add)
                        with nc.allow_low_precision(reason="v in fp8, validated"):
                            nc.vector.tensor_tensor(
                                out=v8h[:, 0:4, 0:D],
                                in0=psM2[:, 256:512].rearrange(
                                    "p (h d) -> p h d", d=D),
                                in1=qkvb_sb[:, 768:1024].rearrange(
                                    "p (h d) -> p h d", d=D),
                                op=AL.add)
                        kn_t = p2w.tile([128, C], BF16, tag="kn_t", name="kn_t")
                        rms_apply(kv_t[:, 0:C], ksc_sb, kn_t, "k")
                        ktr = p2w.tile([128, KT, 128], BF16, tag="ktr", name="ktr")
                        nc.sync.dma_start_transpose(out=ktr, in_=kn_t)
                        k8 = p2w.tile([128, KT, 128], F8, tag="k8", name="k8")
                        with nc.allow_low_precision(reason="fp8 k validated"):
                            nc.vector.tensor_copy(out=k8, in_=ktr)
                        hf, lsl = rt // 4, slice((rt % 4) * 128, (rt % 4 + 1) * 128)
                        nc.sync.dma_start(out=kv_k_view[hf][:, :, lsl], in_=k8)
                        nc.sync.dma_start(out=kv_v_view[hf][lsl, :], in_=v8_t)
                        if rt == 3:
                            emit_gather(0)
                    emit_gather(1)

                    # ---- q side (overlaps the gather) ----
                    for rt in range(RT):
                        q_t = p2w.tile([128, C], BF16, tag="q_t", name="q_t")
                        for c0, cw in ((1536, 512), (2048, 256)):
                            psQ = qkv_mm(rt, c0, cw)
                            nc.vector.tensor_tensor(
                                out=q_t[:, c0 - 1536:c0 - 1536 + cw],
                                in0=psQ[:, 0:cw],
                                in1=qkvb_sb[:, c0:c0 + cw], op=AL.add)
                        qn_t = p2w.tile([128, C], BF16, tag="qn_t", name="qn_t")
                        rms_apply(q_t, qsc_sb, qn_t, "q")
                        qtr = p2w.tile([128, KT, 128], BF16, tag="qtr", name="qtr")
                        nc.sync.dma_start_transpose(out=qtr, in_=qn_t)
                        with nc.allow_low_precision(reason="fp8 q validated"):
                            nc.vector.tensor_copy(
                                out=qT_sb[:, :, rt * 128:(rt + 1) * 128], in_=qtr)

                # ===== P4 prefetch: proj weights stream in under attention ====
                p4pre = ctx.enter_context(tc.tile_pool(name="p4pre", bufs=1))
                pw_sb = p4pre.tile([128, KT, C], BF16)
                nc.scalar.dma_start(
                    out=pw_sb, in_=pw_in.rearrange("(t p) j -> p t j", p=128))
                pb_sb = p4pre.tile([128, C], F32)
                nc.scalar.dma_start(out=pb_sb, in_=pb_in[:, :])

                # ===== P3: attention =====
                if "attn" in ablate:
                    nc.vector.memset(oT_sb, 0.0)
                else:
                  with tc.tile_pool(name="p3v", bufs=1) as p3v, \
                       tc.tile_pool(name="p3b", bufs=2) as p3b, \
                       tc.tile_pool(name="p3k", bufs=4) as p3k, \
                       tc.tile_pool(name="p3a", bufs=4) as p3a, \
                       tc.tile_pool(name="p3r", bufs=2) as p3r, \
                       tc.tile_pool(name="psS", bufs=3, space="PSUM") as psSp, \
                       tc.tile_pool(name="psO", bufs=2, space="PSUM") as psOp:
                    # persistent v' tiles fp8 [128, 64, 784]; ones columns
                    # arrive pre-interleaved from the gather.
                    v_sb = p3v.tile([128, 64, VP], F8)

                    def load_v_tiles(b):
                        for i in range(16):
                            cp, half = i // 2, i % 2
                            r0 = (b % 2) * NS + half * 128
                            src = kv_all[b // 2][cp, KV_H:].rearrange(
                                "(r c) -> r c", c=VP)
                            nc.gpsimd.dma_start(
                                out=v_sb[:, b * 16 + i, :],
                                in_=src[r0:r0 + 128, :])

                    def bias_load(g):
                        bias_g = p3b.tile([128, 2, 16, NS], BF16, tag="bias_g",
                                          name="bias_g")
                        nc.sync.dma_start(
                            out=bias_g,
                            in_=bias_in[2 * g:2 * g + 2].rearrange(
                                "h (i p) n -> p h i n", p=128))
                        return bias_g

                    # two bias groups prefetch on the sync queue during the
                    # gather; v tiles follow once the gather lands.
                    bias_ring = [bias_load(0), bias_load(1)]
                    load_v_tiles(0)
                    kge = [a[:, 0:KV_H].rearrange("c (r n) -> c r n", n=RH)
                           for a in kv_all]
                    for g in range(KT):
                        bias_g = bias_ring[g % 2]
                        for b in range(B):
                            kT2 = p3k.tile([128, NCORE, NS], F8, tag="kT2",
                                           name="kT2")
                            nc.sync.dma_start(
                                out=kT2,
                                in_=kge[b // 2][:, g * 128:(g + 1) * 128,
                                                (b % 2) * NS:
                                                (b % 2 + 1) * NS].rearrange(
                                            "c p n -> p c n"))
                            kT2f = kT2.rearrange("p c n -> p (c n)")
                            if g == 0 and b < B - 1:
                                load_v_tiles(b + 1)
                            if g < KT - 2 and b == B - 1:
                                bias_ring[g % 2] = bias_load(g + 2)
                            for hh in range(2):
                                h = 2 * g + hh
                                pb = hh * 64
                                psO = psOp.tile([128, 2 * NS], F32, tag="psO",
                                                name="psO")
                                for j4 in range(4):
                                    psS = psSp.tile([128, 1024], F32, tag="psS",
                                                    name="psS")
                                    attnT = p3a.tile([128, 1024], F8E5,
                                                     tag="attnT", name="attnT")
                                    for t2 in range(2):
                                        nc.tensor.matmul(
                                            psS[:, t2 * 512:(t2 + 1) * 512],
                                            lhsT=id_sb,
                                            rhs=bias_g[:, hh,
                                                       4 * j4 + 2 * t2:
                                                       4 * j4 + 2 * t2 + 2, :],
                                            start=True, stop=False,
                                            skip_group_check=True)
                                    for t in range(4):
                                        i = 4 * j4 + t
                                        tsl = slice(t * 256, (t + 1) * 256)
                                        nc.tensor.matmul(
                                            psS[:, tsl],
                                            lhsT=kT2f[pb:pb + 64,
                                                      i * 128:(i + 1) * 128],
                                            rhs=qT_sb[pb:pb + 64, g,
                                                      b * NS:(b + 1) * NS],
                                            start=False, stop=True)
                                    with nc.allow_low_precision(
                                            reason="fp8 attn validated 1.5e-3"):
                                        nc.scalar.activation(out=attnT, in_=psS,
                                                             func=AF.Exp)
                                    for pr in range(2):
                                        j = 2 * j4 + pr
                                        nc.tensor.matmul(
                                            psO[0:D + 1, 0:NS],
                                            lhsT=v_sb[:, b * 16 + 2 * j:
                                                      b * 16 + 2 * j + 2,
                                                      h * (D + 1):
                                                      (h + 1) * (D + 1)],
                                            rhs=attnT[:, 2 * pr * 256:
                                                      (2 * pr + 2) * 256].rearrange(
                                                "p (two n) -> p two n", two=2),
                                            start=(j == 0), stop=(j == 7),
                                            perf_mode=mybir.MatmulPerfMode.DoubleRow)
                                rs = p3r.tile([128, NS], BF16, tag="rs", name="rs")
                                with nc.allow_low_precision(
                                        reason="bf16 softmax denom reciprocal, "
                                               "validated 1.2e-3 end-to-end"):
                                    nc.vector.reciprocal(out=rs[64:65, :],
                                                         in_=psO[D:D + 1, 0:NS])
                                nc.tensor.matmul(psO[:, NS:2 * NS],
                                                 lhsT=ones_sb[64:65, :],
                                                 rhs=rs[64:65, :], start=True,
                                                 stop=True, skip_group_check=True)
                                rb = p3r.tile([128, NS], BF16, tag="rb", name="rb")
                                nc.vector.tensor_copy(out=rb[0:D, :],
                                                      in_=psO[0:D, NS:2 * NS])
                                nc.vector.tensor_tensor(
                                    out=oT_sb[pb:pb + 64, g, b * NS:(b + 1) * NS],
                                    in0=psO[0:D, 0:NS], in1=rb[0:D, :], op=AL.mult)

                # ===== P4: proj + residual =====
                with tc.tile_pool(name="p4w", bufs=3) as p4w, \
                     tc.tile_pool(name="p4ps", bufs=3, space="PSUM") as p4ps, \
                     tc.tile_pool(name="bc2", bufs=1) as bc2:
                    g1_bc = []
                    mlp_sc, mlp_sh = [], []
                    for b in range(B):
                        g1 = bc2.tile([128, C], F32, tag=f"g1_{b}", name=f"g1_{b}")
                        nc.sync.dma_start(out=g1, in_=_bc(mod_dram[b, 2 * C:3 * C]))
                        g1_bc.append(g1)
                        sc = bc2.tile([128, C], F32, tag=f"sc2_{b}", name=f"sc2_{b}")
                        nc.sync.dma_start(out=sc, in_=_bc(mod_dram[b, 4 * C:5 * C]))
                        sh = bc2.tile([128, C], F32, tag=f"sh2_{b}", name=f"sh2_{b}")
                        nc.sync.dma_start(out=sh, in_=_bc(mod_dram[b, 3 * C:4 * C]))
                        mlp_sc.append(sc)
                        mlp_sh.append(sh)
                    for rt in range(RT):
                        t1 = p4w.tile([128, C], F32, tag="pj_t1", name="pj_t1")
                        for c0, cw in ((0, 512), (512, 256)):
                            psP = p4ps.tile([128, 512], F32, tag="psP", name="psP")
                            for kt in range(KT):
                                nc.tensor.matmul(
                                    psP[:, 0:cw],
                                    lhsT=oT_sb[:, kt, rt * 128:(rt + 1) * 128],
                                    rhs=pw_sb[:, kt, c0:c0 + cw],
                                    start=(kt == 0), stop=(kt == KT - 1))
                            nc.vector.tensor_tensor(out=t1[:, c0:c0 + cw],
                                                    in0=psP[:, 0:cw],
                                                    in1=pb_sb[:, c0:c0 + cw],
                                                    op=AL.add)
                        t2 = p4w.tile([128, C], F32, tag="pj_t2", name="pj_t2")
                        nc.vector.tensor_tensor(out=t2, in0=t1, in1=g1_bc[rt // 2],
                                                op=AL.mult)
                        nc.vector.tensor_tensor(out=x2_sb[:, rt, :], in0=t2,
                                                in1=x1_sb[:, rt, :], op=AL.add)
                        # LN2 + modulate pipelined right behind each proj tile
                        h2_t = p4w.tile([128, C], BF16, tag="h2_t", name="h2_t")
                        _ln_mod(nc, p4w, x2_sb[:, rt, :], mlp_sc[rt // 2],
                                mlp_sh[rt // 2], h2_t, eps_ln)
                        nc.sync.dma_start_transpose(
                            out=h2T_sb[:, :, rt * 128:(rt + 1) * 128], in_=h2_t)

                # ===== P5: SwiGLU MLP =====
                if "mlp" in ablate:
                    for rt in range(RT):
                        nc.sync.dma_start(out=out_t[rt * 128:(rt + 1) * 128, :],
                                          in_=x2_sb[:, rt, :])
                else:
                  with tc.tile_pool(name="p5z", bufs=1) as p5z:
                    zT_sb = p5z.tile([128, FT, R], BF16)
                    # w2 prefetches on the scalar queue under the z loop
                    w2_sb = p5z.tile([128, FT, C], BF16)
                    nc.scalar.dma_start(
                        out=w2_sb, in_=w2_in.rearrange("(t p) j -> p t j", p=128))
                    w2b_sb = p5z.tile([128, C], F32)
                    nc.scalar.dma_start(out=w2b_sb, in_=w2b_in[:, :])
                    with tc.tile_pool(name="p5w", bufs=3) as p5w, \
                         tc.tile_pool(name="p5ps", bufs=2, space="PSUM") as p5ps:
                        w1g = w1_in.rearrange("(t p) j -> p t j", p=128)
                        w3g = w3_in.rearrange("(t p) j -> p t j", p=128)
                        for fc in range(FT // 2):
                            fsl = slice(fc * 256, (fc + 1) * 256)
                            w1_t = p5w.tile([128, KT, 256], BF16, tag="w1_t",
                                            name="w1_t")
                            nc.gpsimd.dma_start(out=w1_t, in_=w1g[:, :, fsl])
                            w3_t = p5w.tile([128, KT, 256], BF16, tag="w3_t",
                                            name="w3_t")
                            nc.gpsimd.dma_start(out=w3_t, in_=w3g[:, :, fsl])
                            for sub in range(2):
                                ft = fc * 2 + sub
                                ssl = slice(sub * 128, (sub + 1) * 128)
                                for nch in range(2):
                                    nsl = slice(nch * 512, (nch + 1) * 512)
                                    psU = p5ps.tile([128, 512], F32, tag="psU",
                                                    name="psU")
                                    psG = p5ps.tile([128, 512], F32, tag="psG",
                                                    name="psG")
                                    for kt in range(KT):
                                        nc.tensor.matmul(psU, lhsT=w1_t[:, kt, ssl],
                                                         rhs=h2T_sb[:, kt, nsl],
                                                         start=(kt == 0),
                                                         stop=(kt == KT - 1))
                                    for kt in range(KT):
                                        nc.tensor.matmul(psG, lhsT=w3_t[:, kt, ssl],
                                                         rhs=h2T_sb[:, kt, nsl],
                                                         start=(kt == 0),
                                                         stop=(kt == KT - 1))
                                    us = p5w.tile([128, 512], BF16, tag="us",
                                                  name="us")
                                    nc.scalar.activation(out=us, in_=psU,
                                                         func=AF.Silu)
                                    nc.vector.tensor_tensor(out=zT_sb[:, ft, nsl],
                                                            in0=us, in1=psG,
                                                            op=AL.mult)

                    # ---- second half: z @ w2 + gate + residual ----
                    with tc.tile_pool(name="bc4", bufs=1) as bc4, \
                         tc.tile_pool(name="p6w", bufs=3) as p6w, \
                         tc.tile_pool(name="p6ps", bufs=2, space="PSUM") as p6ps:
                        g2_bc = []
                        for b in range(B):
                            g2 = bc4.tile([128, C], F32, tag=f"g2_{b}",
                                          name=f"g2_{b}")
                            nc.sync.dma_start(out=g2,
                                              in_=_bc(mod_dram[b, 5 * C:6 * C]))
                            g2_bc.append(g2)
                        for rt in range(RT):
                            t1 = p6w.tile([128, C], F32, tag="o2_t1", name="o2_t1")
                            for c0, cw in ((0, 512), (512, 256)):
                                psP = p6ps.tile([128, 512], F32, tag="psO2",
                                                name="psO2")
                                for kt in range(FT):
                                    nc.tensor.matmul(
                                        psP[:, 0:cw],
                                        lhsT=zT_sb[:, kt, rt * 128:(rt + 1) * 128],
                                        rhs=w2_sb[:, kt, c0:c0 + cw],
                                        start=(kt == 0), stop=(kt == FT - 1))
                                nc.vector.tensor_tensor(out=t1[:, c0:c0 + cw],
                                                        in0=psP[:, 0:cw],
                                                        in1=w2b_sb[:, c0:c0 + cw],
                                                        op=AL.add)
                            t2 = p6w.tile([128, C], F32, tag="o2_t2", name="o2_t2")
                            nc.vector.tensor_tensor(out=t2, in0=t1,
                                                    in1=g2_bc[rt // 2], op=AL.mult)
                            y_t = p6w.tile([128, C], F32, tag="y_t", name="y_t")
                            nc.vector.tensor_tensor(out=y_t, in0=t2,
                                                    in1=x2_sb[:, rt, :], op=AL.add)
                            nc.sync.dma_start(out=out_t[rt * 128:(rt + 1) * 128, :],
                                              in_=y_t)

    nc.compile()
    return nc


_CACHE = {}


def _get_nc():
    if "nc" not in _CACHE:
        _CACHE["nc"] = build()
    return _CACHE["nc"]


def prepare_in_maps(inputs):
    inputs = {k: np.asarray(v) for k, v in inputs.items()}
    x = inputs["x"].astype(np.float32)
    c = inputs["c"].astype(np.float32)
    bias = inputs["bias"].astype(np.float32)
    q_scale = inputs["q_scale"].astype(np.float32)
    k_scale = inputs["k_scale"].astype(np.float32)

    qkv_w_f = inputs["qkv_w"].astype(np.float32)          # rows [q; k; v]
    qkv_w_kvq = np.concatenate(
        [qkv_w_f[C:2 * C], qkv_w_f[2 * C:], qkv_w_f[:C]], axis=0)
    qkv_wT = np.ascontiguousarray(qkv_w_kvq.T.astype(bf16))
    proj_wT = np.ascontiguousarray(inputs["proj_w"].astype(np.float32).T.astype(bf16))
    w1T = np.ascontiguousarray(inputs["w1"].astype(np.float32).T.astype(bf16))
    w3T = np.ascontiguousarray(inputs["w3"].astype(np.float32).T.astype(bf16))
    w2T = np.ascontiguousarray(inputs["w2_w"].astype(np.float32).T.astype(bf16))
    adaln_wT = np.ascontiguousarray(
        inputs["adaln_w"].astype(np.float32).T.astype(bf16))
    adaln_b4 = np.broadcast_to(
        inputs["adaln_b"].astype(np.float32), (B, 6 * C)).copy()
    adaln_b4[:, C:2 * C] += 1.0      # fold modulate's (1 + scale)
    adaln_b4[:, 4 * C:5 * C] += 1.0
    qkv_b_f = inputs["qkv_b"].astype(np.float32)
    qkv_b_kvq = np.concatenate([qkv_b_f[C:2 * C], qkv_b_f[2 * C:], qkv_b_f[:C]])
    qkv_b_bc = np.ascontiguousarray(np.broadcast_to(qkv_b_kvq, (128, 3 * C)))
    proj_b_bc = np.ascontiguousarray(
        np.broadcast_to(inputs["proj_b"].astype(np.float32), (128, C)))
    w2_b_bc = np.ascontiguousarray(
        np.broadcast_to(inputs["w2_b"].astype(np.float32), (128, C)))
    qscale_bc = np.ascontiguousarray(np.broadcast_to(
        np.tile(q_scale * D ** -0.5, H).astype(bf16), (128, C)))
    kscale_bc = np.ascontiguousarray(np.broadcast_to(
        np.tile(k_scale, H).astype(bf16), (128, C)))
    cT = np.ascontiguousarray(c.T)
    biasT = np.ascontiguousarray(bias[0].transpose(0, 2, 1).astype(bf16))
    id128 = np.eye(128, dtype=bf16)

    in_maps = []
    for cc in range(NCORE):
        sl = slice(cc * NS, (cc + 1) * NS)
        in_maps.append({
            "x": np.ascontiguousarray(x[:, sl, :].reshape(R, C)),
            "cT": cT,
            "bias_t": np.ascontiguousarray(biasT[:, :, sl]),
            "adaln_wT": adaln_wT, "adaln_b4": adaln_b4,
            "qkv_wT": qkv_wT, "qkv_b_bc": qkv_b_bc,
            "qscale_bc": qscale_bc, "kscale_bc": kscale_bc,
            "proj_wT": proj_wT, "proj_b_bc": proj_b_bc,
            "w1T": w1T, "w3T": w3T, "w2T": w2T, "w2_b_bc": w2_b_bc,
            "id128": id128,
        })

    return in_maps


def kernel(**inputs):
    in_maps = prepare_in_maps(inputs)
    nc = _get_nc()
    res = run_bass_kernel_spmd(nc, in_maps, core_ids=list(range(NCORE)))
    _CACHE["last_res"] = res
    out = np.empty((B, N, C), np.float32)
    for cc in range(NCORE):
        out[:, cc * NS:(cc + 1) * NS, :] = res.results[cc]["out"].reshape(B, NS, C)
    return out


# revision 34
# speedup vs baseline: 1.1171x; 1.1171x over previous
"""DiT block kernel for 8 Trainium2 NeuronCores (Bass/Tile).

Sharding: each core owns a 256-wide query slice of the sequence (all batches,
all heads).
 - LN1/modulate/QKV/rmsnorm computed on own rows; K^T (bf16) and V (fp8,
   pre-interleaved with a ones-column per head) are gathered with ONE fused
   AllGather so every core holds full K/V.
 - Attention bias is pre-transposed on host to [H, m, n] and sliced per core
   along n, so every bias element is read exactly once across the machine.
   Bias is pre-loaded into PSUM via an identity matmul, the scores matmul
   accumulates on top, ScalarE applies exp (PSUM -> SBUF fp8e5m2).
 - o^T accumulated on PE in fp8 DoubleRow over m-tile pairs, with the ones
   column so the softmax denominator rides along as psum row 64; the divide
   is folded into the o^T evacuation via a tiny broadcast matmul.
 - proj/MLP are row-local; outputs concatenated on host.

Engine balance: per-head rmsnorm scaling runs on the Pool engine; LN1 stats
overlap the adaLN matmuls; proj/w2 weights prefetch under attention/MLP.
"""

import contextlib

import numpy as np
import ml_dtypes

import concourse.bacc as bacc
import concourse.tile as tile
from concourse import mybir
from concourse.bass_utils import run_bass_kernel_spmd

bf16 = ml_dtypes.bfloat16
F32 = mybir.dt.float32
BF16 = mybir.dt.bfloat16
F8 = mybir.dt.float8e4
F8E5 = mybir.dt.float8e5
AF = mybir.ActivationFunctionType
AL = mybir.AluOpType

B, N, C = 4, 2048, 768
H, D = 12, 64
FFN = 2048
NCORE = 8
NS = N // NCORE          # 256 queries per core
R = B * NS               # 1024 rows per core
RT = R // 128            # 8 row tiles
KT = C // 128            # 6 contraction tiles over C
FT = FFN // 128          # 16 FFN row tiles
EPS_LN, EPS_RMS = 1e-6, 1e-8

VW = H * (D + 1)         # 780: v row width with per-head ones column
VP = 784                 # padded v row stride (fp8 DoubleRow ldweights needs
                         # 16B-aligned tile strides)
KV_K = C * R             # elems (= bytes, fp8) of k^T shard block
KV_TOT = KV_K + R * VP   # fused shard bytes: k^T | v'


def _bc(ap, parts=128):
    """partition-stride-0 broadcast AP (DRAM source)."""
    import dataclasses
    return dataclasses.replace(ap, ap=[[0, parts]] + list(ap.ap))


def _ln_mod(nc, pool, src_ap, sc_bc, sh_bc, dst_bf, eps_tile):
    """dst = LN(src) * sc + sh   (sc already includes the +1)."""
    stats = pool.tile([128, 2, 6], F32, tag="ln_stats", name="ln_stats")
    nc.vector.bn_stats(out=stats[:, 0, :], in_=src_ap[:, 0:384])
    nc.vector.bn_stats(out=stats[:, 1, :], in_=src_ap[:, 384:768])
    mv = pool.tile([128, 2], F32, tag="ln_mv", name="ln_mv")
    nc.vector.bn_aggr(out=mv, in_=stats)
    rstd = pool.tile([128, 1], F32, tag="ln_rstd", name="ln_rstd")
    nc.scalar.activation(out=rstd, in_=mv[:, 1:2], func=AF.Sqrt, bias=eps_tile)
    nc.vector.reciprocal(out=rstd, in_=rstd)
    t1 = pool.tile([128, C], F32, tag="ln_t1", name="ln_t1")
    nc.vector.tensor_scalar(out=t1, in0=src_ap, scalar1=mv[:, 0:1], scalar2=rstd,
                            op0=AL.subtract, op1=AL.mult)
    nc.vector.tensor_tensor(out=t1, in0=t1, in1=sc_bc, op=AL.mult)
    nc.vector.tensor_tensor(out=dst_bf, in0=t1, in1=sh_bc, op=AL.add)


def build(collective=True, repeat=1, ablate=()):
    ablate = frozenset(ablate)
    nc = bacc.Bacc("TRN2", target_bir_lowering=False, debug=False,
                   num_devices=NCORE)

    x_in = nc.dram_tensor("x", [R, C], F32, kind="ExternalInput")
    cT_in = nc.dram_tensor("cT", [C, B], F32, kind="ExternalInput")
    bias_in = nc.dram_tensor("bias_t", [H, N, NS], BF16, kind="ExternalInput")
    adw_in = nc.dram_tensor("adaln_wT", [C, 6 * C], BF16, kind="ExternalInput")
    adb_in = nc.dram_tensor("adaln_b4", [B, 6 * C], F32, kind="ExternalInput")
    qkvw_in = nc.dram_tensor("qkv_wT", [C, 3 * C], BF16, kind="ExternalInput")
    qkvb_in = nc.dram_tensor("qkv_b_bc", [128, 3 * C], F32, kind="ExternalInput")
    qsc_in = nc.dram_tensor("qscale_bc", [128, C], BF16, kind="ExternalInput")
    ksc_in = nc.dram_tensor("kscale_bc", [128, C], BF16, kind="ExternalInput")
    pw_in = nc.dram_tensor("proj_wT", [C, C], BF16, kind="ExternalInput")
    pb_in = nc.dram_tensor("proj_b_bc", [128, C], F32, kind="ExternalInput")
    w1_in = nc.dram_tensor("w1T", [C, FFN], BF16, kind="ExternalInput")
    w3_in = nc.dram_tensor("w3T", [C, FFN], BF16, kind="ExternalInput")
    w2_in = nc.dram_tensor("w2T", [FFN, C], BF16, kind="ExternalInput")
    w2b_in = nc.dram_tensor("w2_b_bc", [128, C], F32, kind="ExternalInput")
    id_in = nc.dram_tensor("id128", [128, 128], BF16, kind="ExternalInput")
    out_t = nc.dram_tensor("out", [R, C], F32, kind="ExternalOutput")

    with tile.TileContext(nc, num_cores=NCORE) as tc, contextlib.ExitStack() as top:
        consts = top.enter_context(tc.tile_pool(name="consts", bufs=1))
        dram = top.enter_context(tc.tile_pool(name="dram", bufs=1, space="DRAM"))
        keep = top.enter_context(tc.tile_pool(name="keep", bufs=1))

        eps_ln = consts.tile([128, 1], F32)
        nc.vector.memset(eps_ln, EPS_LN)
        id_sb = consts.tile([128, 128], BF16)
        nc.sync.dma_start(out=id_sb, in_=id_in[:, :])
        ones_sb = consts.tile([128, 128], BF16)
        nc.vector.memset(ones_sb, 1.0)

        for _rep in range(repeat):
            with contextlib.ExitStack() as ctx:
                qT_sb = keep.tile([128, KT, R], F8)       # packed q^T (fp8)
                oT_sb = keep.tile([128, KT, R], BF16)     # packed normalized o^T
                h2T_sb = keep.tile([128, KT, R], BF16)    # packed LN2-mod x2^T
                mod_dram = dram.tile([B, 6 * C], F32)
                x1_sb = keep.tile([128, RT, C], F32)      # resident input rows
                x2_sb = keep.tile([128, RT, C], F32)      # post-attn residual
                st_sb = keep.tile([128, RT, 2, 6], F32)   # LN1 bn stats
                mv_sb = keep.tile([128, RT, 2], F32)      # LN1 mean/var
                rs_sb = keep.tile([128, RT, 1], F32)      # LN1 rstd

                # ===== P0: adaLN modulation, overlapped with LN1 stats ======
                with tc.tile_pool(name="p0", bufs=1) as p0, \
                     tc.tile_pool(name="p0c", bufs=3) as p0c, \
                     tc.tile_pool(name="p0ps", bufs=2, space="PSUM") as p0ps:
                    cT_sb = p0.tile([128, KT, B], F32)
                    nc.sync.dma_start(
                        out=cT_sb, in_=cT_in.rearrange("(t p) b -> p t b", p=128))
                    scT = p0.tile([128, KT, B], BF16)
                    nc.scalar.activation(out=scT, in_=cT_sb, func=AF.Silu)

                    # LN1 stats for all row tiles: x DMA on gpsimd queue, stats
                    # on DVE — both run under the adaLN matmuls below.
                    for rt in range(RT):
                        nc.gpsimd.dma_start(
                            out=x1_sb[:, rt, :],
                            in_=x_in[rt * 128:(rt + 1) * 128, :])
                        nc.vector.bn_stats(out=st_sb[:, rt, 0, :],
                                           in_=x1_sb[:, rt, 0:384])
                        nc.vector.bn_stats(out=st_sb[:, rt, 1, :],
                                           in_=x1_sb[:, rt, 384:768])
                        nc.vector.bn_aggr(out=mv_sb[:, rt], in_=st_sb[:, rt])
                        nc.scalar.activation(out=rs_sb[:, rt], in_=mv_sb[:, rt, 1:2],
                                             func=AF.Sqrt, bias=eps_ln)
                        nc.vector.reciprocal(out=rs_sb[:, rt], in_=rs_sb[:, rt])

                    adwg = adw_in.rearrange("(t p) j -> p t j", p=128)
                    adb_sb = p0.tile([B, 6 * C], F32)
                    nc.sync.dma_start(out=adb_sb, in_=adb_in[:, :])
                    mod_sb = p0.tile([B, 6 * C], F32)
                    for big in range(3):
                        bsl = slice(big * 1536, (big + 1) * 1536)
                        adw_t = p0c.tile([128, KT, 1536], BF16, tag="adw_t",
                                         name="adw_t")
                        nc.sync.dma_start(out=adw_t, in_=adwg[:, :, bsl])
                        for sub in range(3):
                            c0 = big * 1536 + sub * 512
                            sl = slice(c0, c0 + 512)
                            psM = p0ps.tile([B, 512], F32, tag="psM", name="psM")
                            for kt in range(KT):
                                nc.tensor.matmul(psM, lhsT=scT[:, kt, :],
                                                 rhs=adw_t[:, kt,
                                                           sub * 512:(sub + 1) * 512],
                                                 start=(kt == 0), stop=(kt == KT - 1))
                            nc.vector.tensor_tensor(out=mod_sb[:, sl], in0=psM,
                                                    in1=adb_sb[:, sl], op=AL.add)
                        nc.gpsimd.dma_start(out=mod_dram[:, bsl], in_=mod_sb[:, bsl])

                # ===== P1+P2: LN1 modulate, QKV (k,v first), rmsnorm =====
                # qkv weight columns are host-permuted to [k | v | q] so the K/V
                # side finishes first and the AllGather overlaps Q-side compute.
                # two half-shards (batches 0-1 | 2-3): the first AllGather
                # fires mid-way through the k/v loop and overlaps the rest of
                # P2; attention on b0/b1 overlaps the second gather.
                RH = R // 2
                KV_H = KT * 128 * RH
                TOT_H = KV_H + RH * VP
                addr = "Shared" if collective else "Local"
                kv_shard = [dram.tile([1, TOT_H], F8, tag=f"kvs{h}",
                                      name=f"kvs{h}") for h in range(2)]
                kv_all = [dram.tile([NCORE, TOT_H], F8, addr_space=addr,
                                    tag=f"kva{h}", name=f"kva{h}")
                          for h in range(2)]
                kv_k_view = [s[0, 0:KV_H].rearrange("(t p n) -> p t n",
                                                    p=128, t=KT)
                             for s in kv_shard]
                kv_v_view = [s[0, KV_H:].rearrange("(r c) -> r c", c=VP)
                             for s in kv_shard]

                def emit_gather(h):
                    if "ag" in ablate:
                        return
                    if collective:
                        nc.gpsimd.collective_compute(
                            "AllGather", AL.bypass,
                            replica_groups=[list(range(NCORE))],
                            ins=[kv_shard[h].opt()], outs=[kv_all[h].opt()],
                        )
                    else:
                        for cc in range(2):
                            nc.scalar.dma_start(out=kv_all[h][cc:cc + 1, :],
                                                in_=kv_shard[h][:, :])
                with tc.tile_pool(name="bc1", bufs=1) as bc1, \
                     tc.tile_pool(name="p2", bufs=1) as p2, \
                     tc.tile_pool(name="p2w", bufs=2) as p2w, \
                     tc.tile_pool(name="p2ps", bufs=4, space="PSUM") as p2ps:
                    msa_sc, msa_sh = [], []
                    for b in range(B):
                        # host folded the +1 into adaln_b4's scale segments
                        sc = bc1.tile([128, C], F32, tag=f"sc1_{b}", name=f"sc1_{b}")
                        nc.sync.dma_start(out=sc, in_=_bc(mod_dram[b, C:2 * C]))
                        sh = bc1.tile([128, C], F32, tag=f"sh1_{b}", name=f"sh1_{b}")
                        nc.sync.dma_start(out=sh, in_=_bc(mod_dram[b, 0:C]))
                        msa_sc.append(sc)
                        msa_sh.append(sh)

                    qkvw_sb = p2.tile([128, KT, 3 * C], BF16)
                    nc.scalar.dma_start(
                        out=qkvw_sb, in_=qkvw_in.rearrange("(t p) j -> p t j", p=128))
                    qkvb_sb = p2.tile([128, 3 * C], F32)
                    nc.scalar.dma_start(out=qkvb_sb, in_=qkvb_in[:, :])
                    qsc_sb = p2.tile([128, C], BF16)
                    nc.scalar.dma_start(out=qsc_sb, in_=qsc_in[:, :])
                    ksc_sb = p2.tile([128, C], BF16)
                    nc.scalar.dma_start(out=ksc_sb, in_=ksc_in[:, :])

                    h1T_sb = p2.tile([128, KT, R], BF16)
                    for rt in range(RT):
                        t1 = p2w.tile([128, C], F32, tag="m1_t1", name="m1_t1")
                        nc.vector.tensor_scalar(
                            out=t1, in0=x1_sb[:, rt, :], scalar1=mv_sb[:, rt, 0:1],
                            scalar2=rs_sb[:, rt], op0=AL.subtract, op1=AL.mult)
                        nc.vector.tensor_tensor(out=t1, in0=t1,
                                                in1=msa_sc[rt // 2], op=AL.mult)
                        h1_t = p2w.tile([128, C], BF16, tag="h1_t", name="h1_t")
                        nc.vector.tensor_tensor(out=h1_t, in0=t1,
                                                in1=msa_sh[rt // 2], op=AL.add)
                        nc.sync.dma_start_transpose(
                            out=h1T_sb[:, :, rt * 128:(rt + 1) * 128], in_=h1_t)

                    def qkv_mm(rt, c0, cw):
                        psQ = p2ps.tile([128, 512], F32, tag="psQ", name="psQ")
                        for kt in range(KT):
                            nc.tensor.matmul(
                                psQ[:, 0:cw],
                                lhsT=h1T_sb[:, kt, rt * 128:(rt + 1) * 128],
                                rhs=qkvw_sb[:, kt, c0:c0 + cw],
                                start=(kt == 0), stop=(kt == KT - 1))
                        return psQ

                    def pool_headmul(dst, ss):
                        for h in range(H):
                            hs = slice(h * D, (h + 1) * D)
                            nc.gpsimd.tensor_scalar(
                                out=dst[:, hs], in0=dst[:, hs],
                                scalar1=ss[:, h:h + 1], scalar2=None, op0=AL.mult)

                    def rms_apply(t, scale_sb, dst, tagp):
                        """t: [128, 768] bf16 -> dst normalized bf16."""
                        sq = p2w.tile([128, C], BF16, tag=f"sq{tagp}", name="sq")
                        nc.vector.tensor_tensor(out=sq, in0=t, in1=t, op=AL.mult)
                        ss = p2w.tile([128, H], F32, tag=f"ss{tagp}", name="ss")
                        nc.vector.tensor_reduce(
                            out=ss, in_=sq.rearrange("p (h d) -> p h d", d=D),
                            axis=mybir.AxisListType.X, op=AL.add)
                        nc.scalar.activation(out=ss, in_=ss, func=AF.Sqrt,
                                             scale=1.0 / D)
                        nc.vector.tensor_scalar_add(out=ss, in0=ss, scalar1=EPS_RMS)
                        nc.vector.reciprocal(out=ss, in_=ss)
                        nc.vector.tensor_tensor(out=dst, in0=t, in1=scale_sb,
                                                op=AL.mult)
                        pool_headmul(dst, ss)

                    # ---- k,v side ----
                    for rt in range(RT):
                        rsl = slice(rt * 128, (rt + 1) * 128)
                        kv_t = p2w.tile([128, C], BF16, tag="kv_t", name="kv_t")
                        v8_t = p2w.tile([128, VP], F8, tag="v8_t", name="v8_t")
                        v8h = v8_t[:, 0:VW].rearrange("p (h e) -> p h e", e=D + 1)
                        nc.vector.memset(v8h[:, :, D:D + 1], 1.0)
                        nc.vector.memset(v8_t[:, VW:VP], 0.0)
                        psK = qkv_mm(rt, 0, 512)
                        nc.vector.tensor_tensor(
                            out=kv_t[:, 0:512], in0=psK,
                            in1=qkvb_sb[:, 0:512], op=AL.add)
                        psV = qkv_mm(rt, 1024, 512)
                        with nc.allow_low_precision(reason="fp8 v path validated"):
                            nc.vector.tensor_tensor(
                                out=v8h[:, 4:12, 0:D],
                                in0=psV.rearrange("p (h d) -> p h d", d=D),
                                in1=qkvb_sb[:, 1024:1536].rearrange(
                                    "p (h d) -> p h d", d=D),
                                op=AL.add)
                        # middle chunk straddles k|v: split the evacuation
                        psM2 = qkv_mm(rt, 512, 512)
                        nc.vector.tensor_tensor(
                            out=kv_t[:, 512:768], in0=psM2[:, 0:256],
                            in1=qkvb_sb[:, 512:768], op=AL.add)
                        with nc.allow_low_precision(reason="v in fp8, validated"):
                            nc.vector.tensor_tensor(
                                out=v8h[:, 0:4, 0:D],
                                in0=psM2[:, 256:512].rearrange(
                                    "p (h d) -> p h d", d=D),
                                in1=qkvb_sb[:, 768:1024].rearrange(
                                    "p (h d) -> p h d", d=D),
                                op=AL.add)
                        kn_t = p2w.tile([128, C], BF16, tag="kn_t", name="kn_t")
                        rms_apply(kv_t[:, 0:C], ksc_sb, kn_t, "k")
                        ktr = p2w.tile([128, KT, 128], BF16, tag="ktr", name="ktr")
                        nc.sync.dma_start_transpose(out=ktr, in_=kn_t)
                        k8 = p2w.tile([128, KT, 128], F8, tag="k8", name="k8")
                        with nc.allow_low_precision(reason="fp8 k validated"):
                            nc.vector.tensor_copy(out=k8, in_=ktr)
                        hf, lsl = rt // 4, slice((rt % 4) * 128, (rt % 4 + 1) * 128)
                        nc.sync.dma_start(out=kv_k_view[hf][:, :, lsl], in_=k8)
                        nc.sync.dma_start(out=kv_v_view[hf][lsl, :], in_=v8_t)
                        if rt == 3:
                            emit_gather(0)
                    emit_gather(1)

                    # ---- q side (overlaps the gather) ----
                    for rt in range(RT):
                        q_t = p2w.tile([128, C], BF16, tag="q_t", name="q_t")
                        for c0, cw in ((1536, 512), (2048, 256)):
                            psQ = qkv_mm(rt, c0, cw)
                            nc.vector.tensor_tensor(
                                out=q_t[:, c0 - 1536:c0 - 1536 + cw],
                                in0=psQ[:, 0:cw],
                                in1=qkvb_sb[:, c0:c0 + cw], op=AL.add)
                        qn_t = p2w.tile([128, C], BF16, tag="qn_t", name="qn_t")
                        rms_apply(q_t, qsc_sb, qn_t, "q")
                        qtr = p2w.tile([128, KT, 128], BF16, tag="qtr", name="qtr")
                        nc.sync.dma_start_transpose(out=qtr, in_=qn_t)
                        with nc.allow_low_precision(reason="fp8 q validated"):
                            nc.vector.tensor_copy(
                                out=qT_sb[:, :, rt * 128:(rt + 1) * 128], in_=qtr)

                # ===== P4 prefetch: proj weights stream in under attention ====
                p4pre = ctx.enter_context(tc.tile_pool(name="p4pre", bufs=1))
                pw_sb = p4pre.tile([128, KT, C], BF16)
                nc.scalar.dma_start(
                    out=pw_sb, in_=pw_in.rearrange("(t p) j -> p t j", p=128))
                pb_sb = p4pre.tile([128, C], F32)
                nc.scalar.dma_start(out=pb_sb, in_=pb_in[:, :])

                # ===== P3: attention =====
                if "attn" in ablate:
                    nc.vector.memset(oT_sb, 0.0)
                else:
                  with tc.tile_pool(name="p3v", bufs=1) as p3v, \
                       tc.tile_pool(name="p3b", bufs=2) as p3b, \
                       tc.tile_pool(name="p3k", bufs=4) as p3k, \
                       tc.tile_pool(name="p3a", bufs=4) as p3a, \
                       tc.tile_pool(name="p3r", bufs=2) as p3r, \
                       tc.tile_pool(name="psS", bufs=3, space="PSUM") as psSp, \
                       tc.tile_pool(name="psO", bufs=2, space="PSUM") as psOp:
                    # persistent v' tiles fp8 [128, 64, 784]; ones columns
                    # arrive pre-interleaved from the gather.
                    v_sb = p3v.tile([128, 64, VP], F8)

                    def load_v_tiles(b):
                        for i in range(16):
                            cp, half = i // 2, i % 2
                            r0 = (b % 2) * NS + half * 128
                            src = kv_all[b // 2][cp, KV_H:].rearrange(
                                "(r c) -> r c", c=VP)
                            nc.gpsimd.dma_start(
                                out=v_sb[:, b * 16 + i, :],
                                in_=src[r0:r0 + 128, :])

                    def bias_load(g):
                        bias_g = p3b.tile([128, 2, 16, NS], BF16, tag="bias_g",
                                          name="bias_g")
                        nc.sync.dma_start(
                            out=bias_g,
                            in_=bias_in[2 * g:2 * g + 2].rearrange(
                                "h (i p) n -> p h i n", p=128))
                        return bias_g

                    # two bias groups prefetch on the sync queue during the
                    # gather; v tiles follow once the gather lands.
                    bias_ring = [bias_load(0), bias_load(1)]
                    load_v_tiles(0)
                    kge = [a[:, 0:KV_H].rearrange("c (r n) -> c r n", n=RH)
                           for a in kv_all]
                    for g in range(KT):
                        bias_g = bias_ring[g % 2]
                        for b in range(B):
                            kT2 = p3k.tile([128, NCORE, NS], F8, tag="kT2",
                                           name="kT2")
                            nc.sync.dma_start(
                                out=kT2,
                                in_=kge[b // 2][:, g * 128:(g + 1) * 128,
                                                (b % 2) * NS:
                                                (b % 2 + 1) * NS].rearrange(
                                            "c p n -> p c n"))
                            kT2f = kT2.rearrange("p c n -> p (c n)")
                            if g == 0 and b < B - 1:
                                load_v_tiles(b + 1)
                            if g < KT - 2 and b == B - 1:
                                bias_ring[g % 2] = bias_load(g + 2)
                            for hh in range(2):
                                h = 2 * g + hh
                                pb = hh * 64
                                psO = psOp.tile([128, 2 * NS], F32, tag="psO",
                                                name="psO")
                                for j4 in range(4):
                                    psS = psSp.tile([128, 1024], F32, tag="psS",
                                                    name="psS")
                                    attnT = p3a.tile([128, 1024], F8E5,
                                                     tag="attnT", name="attnT")
                                    for t2 in range(2):
                                        nc.tensor.matmul(
                                            psS[:, t2 * 512:(t2 + 1) * 512],
                                            lhsT=id_sb,
                                            rhs=bias_g[:, hh,
                                                       4 * j4 + 2 * t2:
                                                       4 * j4 + 2 * t2 + 2, :],
                                            start=True, stop=False,
                                            skip_group_check=True)
                                    for t in range(4):
                                        i = 4 * j4 + t
                                        tsl = slice(t * 256, (t + 1) * 256)
                                        nc.tensor.matmul(
                                            psS[:, tsl],
                                            lhsT=kT2f[pb:pb + 64,
                                                      i * 128:(i + 1) * 128],
                                            rhs=qT_sb[pb:pb + 64, g,
                                                      b * NS:(b + 1) * NS],
                                            start=False, stop=True)
                                    with nc.allow_low_precision(
                                            reason="fp8 attn validated 1.5e-3"):
                                        nc.scalar.activation(out=attnT, in_=psS,
                                                             func=AF.Exp)
                                    for pr in range(2):
                                        j = 2 * j4 + pr
                                        nc.tensor.matmul(
                                            psO[0:D + 1, 0:NS],
                                            lhsT=v_sb[:, b * 16 + 2 * j:
                                                      b * 16 + 2 * j + 2,
                                                      h * (D + 1):
                                                      (h + 1) * (D + 1)],
                                            rhs=attnT[:, 2 * pr * 256:
                                                      (2 * pr + 2) * 256].rearrange(
                                                "p (two n) -> p two n", two=2),
                                            start=(j == 0), stop=(j == 7),
                                            perf_mode=mybir.MatmulPerfMode.DoubleRow)
                                rs = p3r.tile([128, NS], BF16, tag="rs", name="rs")
                                with nc.allow_low_precision(
                                        reason="bf16 softmax denom reciprocal, "
                                               "validated 1.2e-3 end-to-end"):
                                    nc.vector.reciprocal(out=rs[64:65, :],
                                                         in_=psO[D:D + 1, 0:NS])
                                nc.tensor.matmul(psO[:, NS:2 * NS],
                                                 lhsT=ones_sb[64:65, :],
                                                 rhs=rs[64:65, :], start=True,
                                                 stop=True, skip_group_check=True)
                                rb = p3r.tile([128, NS], BF16, tag="rb", name="rb")
                                nc.vector.tensor_copy(out=rb[0:D, :],
                                                      in_=psO[0:D, NS:2 * NS])
                                nc.vector.tensor_tensor(
                                    out=oT_sb[pb:pb + 64, g, b * NS:(b + 1) * NS],
                                    in0=psO[0:D, 0:NS], in1=rb[0:D, :], op=AL.mult)

                # ===== P4: proj + residual =====
                with tc.tile_pool(name="p4w", bufs=3) as p4w, \
                     tc.tile_pool(name="p4ps", bufs=3, space="PSUM") as p4ps, \
                     tc.tile_pool(name="bc2", bufs=1) as bc2:
                    g1_bc = []
                    mlp_sc, mlp_sh = [], []
                    for b in range(B):
                        g1 = bc2.tile([128, C], F32, tag=f"g1_{b}", name=f"g1_{b}")
                        nc.sync.dma_start(out=g1, in_=_bc(mod_dram[b, 2 * C:3 * C]))
                        g1_bc.append(g1)
                        sc = bc2.tile([128, C], F32, tag=f"sc2_{b}", name=f"sc2_{b}")
                        nc.sync.dma_start(out=sc, in_=_bc(mod_dram[b, 4 * C:5 * C]))
                        sh = bc2.tile([128, C], F32, tag=f"sh2_{b}", name=f"sh2_{b}")
                        nc.sync.dma_start(out=sh, in_=_bc(mod_dram[b, 3 * C:4 * C]))
                        mlp_sc.append(sc)
                        mlp_sh.append(sh)
                    for rt in range(RT):
                        t1 = p4w.tile([128, C], F32, tag="pj_t1", name="pj_t1")
                        for c0, cw in ((0, 512), (512, 256)):
                            psP = p4ps.tile([128, 512], F32, tag="psP", name="psP")
                            for kt in range(KT):
                                nc.tensor.matmul(
                                    psP[:, 0:cw],
                                    lhsT=oT_sb[:, kt, rt * 128:(rt + 1) * 128],
                                    rhs=pw_sb[:, kt, c0:c0 + cw],
                                    start=(kt == 0), stop=(kt == KT - 1))
                            nc.vector.tensor_tensor(out=t1[:, c0:c0 + cw],
                                                    in0=psP[:, 0:cw],
                                                    in1=pb_sb[:, c0:c0 + cw],
                                                    op=AL.add)
                        t2 = p4w.tile([128, C], F32, tag="pj_t2", name="pj_t2")
                        nc.vector.tensor_tensor(out=t2, in0=t1, in1=g1_bc[rt // 2],
                                                op=AL.mult)
                        nc.vector.tensor_tensor(out=x2_sb[:, rt, :], in0=t2,
                                                in1=x1_sb[:, rt, :], op=AL.add)
                        # LN2 + modulate pipelined right behind each proj tile
                        h2_t = p4w.tile([128, C], BF16, tag="h2_t", name="h2_t")
                        _ln_mod(nc, p4w, x2_sb[:, rt, :], mlp_sc[rt // 2],
                                mlp_sh[rt // 2], h2_t, eps_ln)
                        nc.sync.dma_start_transpose(
                            out=h2T_sb[:, :, rt * 128:(rt + 1) * 128], in_=h2_t)

                # ===== P5: SwiGLU MLP =====
                if "mlp" in ablate:
                    for rt in range(RT):
                        nc.sync.dma_start(out=out_t[rt * 128:(rt + 1) * 128, :],
                                          in_=x2_sb[:, rt, :])
                else:
                  with tc.tile_pool(name="p5z", bufs=1) as p5z:
                    zT_sb = p5z.tile([128, FT, R], BF16)
                    # w2 prefetches on the scalar queue under the z loop
                    w2_sb = p5z.tile([128, FT, C], BF16)
                    nc.scalar.dma_start(
                        out=w2_sb, in_=w2_in.rearrange("(t p) j -> p t j", p=128))
                    w2b_sb = p5z.tile([128, C], F32)
                    nc.scalar.dma_start(out=w2b_sb, in_=w2b_in[:, :])
                    with tc.tile_pool(name="p5w", bufs=3) as p5w, \
                         tc.tile_pool(name="p5ps", bufs=2, space="PSUM") as p5ps:
                        w1g = w1_in.rearrange("(t p) j -> p t j", p=128)
                        w3g = w3_in.rearrange("(t p) j -> p t j", p=128)
                        for fc in range(FT // 2):
                            fsl = slice(fc * 256, (fc + 1) * 256)
                            w1_t = p5w.tile([128, KT, 256], BF16, tag="w1_t",
                                            name="w1_t")
                            nc.sync.dma_start(out=w1_t, in_=w1g[:, :, fsl])
                            w3_t = p5w.tile([128, KT, 256], BF16, tag="w3_t",
                                            name="w3_t")
                            nc.sync.dma_start(out=w3_t, in_=w3g[:, :, fsl])
                            for sub in range(2):
                                ft = fc * 2 + sub
                                ssl = slice(sub * 128, (sub + 1) * 128)
                                for nch in range(2):
                                    nsl = slice(nch * 512, (nch + 1) * 512)
                                    psU = p5ps.tile([128, 512], F32, tag="psU",
                                                    name="psU")
                                    psG = p5ps.tile([128, 512], F32, tag="psG",
                                                    name="psG")
                                    for kt in range(KT):
                                        nc.tensor.matmul(psU, lhsT=w1_t[:, kt, ssl],
                                                         rhs=h2T_sb[:, kt, nsl],
                                                         start=(kt == 0),
                                                         stop=(kt == KT - 1))
                                    for kt in range(KT):
                                        nc.tensor.matmul(psG, lhsT=w3_t[:, kt, ssl],
                                                         rhs=h2T_sb[:, kt, nsl],
                                                         start=(kt == 0),
                                                         stop=(kt == KT - 1))
                                    us = p5w.tile([128, 512], BF16, tag="us",
                                                  name="us")
                                    nc.scalar.activation(out=us, in_=psU,
                                                         func=AF.Silu)
                                    nc.vector.tensor_tensor(out=zT_sb[:, ft, nsl],
                                                            in0=us, in1=psG,
                                                            op=AL.mult)

                    # ---- second half: z @ w2 + gate + residual ----
                    with tc.tile_pool(name="bc4", bufs=1) as bc4, \
                         tc.tile_pool(name="p6w", bufs=3) as p6w, \
                         tc.tile_pool(name="p6ps", bufs=2, space="PSUM") as p6ps:
                        g2_bc = []
                        for b in range(B):
                            g2 = bc4.tile([128, C], F32, tag=f"g2_{b}",
                                          name=f"g2_{b}")
                            nc.sync.dma_start(out=g2,
                                              in_=_bc(mod_dram[b, 5 * C:6 * C]))
                            g2_bc.append(g2)
                        for rt in range(RT):
                            t1 = p6w.tile([128, C], F32, tag="o2_t1", name="o2_t1")
                            for c0, cw in ((0, 512), (512, 256)):
                                psP = p6ps.tile([128, 512], F32, tag="psO2",
                                                name="psO2")
                                for kt in range(FT):
                                    nc.tensor.matmul(
                                        psP[:, 0:cw],
                                        lhsT=zT_sb[:, kt, rt * 128:(rt + 1) * 128],
                                        rhs=w2_sb[:, kt, c0:c0 + cw],
                                        start=(kt == 0), stop=(kt == FT - 1))
                                nc.vector.tensor_tensor(out=t1[:, c0:c0 + cw],
                                                        in0=psP[:, 0:cw],
                                                        in1=w2b_sb[:, c0:c0 + cw],
                                                        op=AL.add)
                            t2 = p6w.tile([128, C], F32, tag="o2_t2", name="o2_t2")
                            nc.vector.tensor_tensor(out=t2, in0=t1,
                                                    in1=g2_bc[rt // 2], op=AL.mult)
                            y_t = p6w.tile([128, C], F32, tag="y_t", name="y_t")
                            nc.vector.tensor_tensor(out=y_t, in0=t2,
                                                    in1=x2_sb[:, rt, :], op=AL.add)
                            nc.sync.dma_start(out=out_t[rt * 128:(rt + 1) * 128, :],
                                              in_=y_t)

    nc.compile()
    return nc


_CACHE = {}


def _get_nc():
    if "nc" not in _CACHE:
        _CACHE["nc"] = build()
    return _CACHE["nc"]


def prepare_in_maps(inputs):
    inputs = {k: np.asarray(v) for k, v in inputs.items()}
    x = inputs["x"].astype(np.float32)
    c = inputs["c"].astype(np.float32)
    bias = inputs["bias"].astype(np.float32)
    q_scale = inputs["q_scale"].astype(np.float32)
    k_scale = inputs["k_scale"].astype(np.float32)

    qkv_w_f = inputs["qkv_w"].astype(np.float32)          # rows [q; k; v]
    qkv_w_kvq = np.concatenate(
        [qkv_w_f[C:2 * C], qkv_w_f[2 * C:], qkv_w_f[:C]], axis=0)
    qkv_wT = np.ascontiguousarray(qkv_w_kvq.T.astype(bf16))
    proj_wT = np.ascontiguousarray(inputs["proj_w"].astype(np.float32).T.astype(bf16))
    w1T = np.ascontiguousarray(inputs["w1"].astype(np.float32).T.astype(bf16))
    w3T = np.ascontiguousarray(inputs["w3"].astype(np.float32).T.astype(bf16))
    w2T = np.ascontiguousarray(inputs["w2_w"].astype(np.float32).T.astype(bf16))
    adaln_wT = np.ascontiguousarray(
        inputs["adaln_w"].astype(np.float32).T.astype(bf16))
    adaln_b4 = np.broadcast_to(
        inputs["adaln_b"].astype(np.float32), (B, 6 * C)).copy()
    adaln_b4[:, C:2 * C] += 1.0      # fold modulate's (1 + scale)
    adaln_b4[:, 4 * C:5 * C] += 1.0
    qkv_b_f = inputs["qkv_b"].astype(np.float32)
    qkv_b_kvq = np.concatenate([qkv_b_f[C:2 * C], qkv_b_f[2 * C:], qkv_b_f[:C]])
    qkv_b_bc = np.ascontiguousarray(np.broadcast_to(qkv_b_kvq, (128, 3 * C)))
    proj_b_bc = np.ascontiguousarray(
        np.broadcast_to(inputs["proj_b"].astype(np.float32), (128, C)))
    w2_b_bc = np.ascontiguousarray(
        np.broadcast_to(inputs["w2_b"].astype(np.float32), (128, C)))
    qscale_bc = np.ascontiguousarray(np.broadcast_to(
        np.tile(q_scale * D ** -0.5, H).astype(bf16), (128, C)))
    kscale_bc = np.ascontiguousarray(np.broadcast_to(
        np.tile(k_scale, H).astype(bf16), (128, C)))
    cT = np.ascontiguousarray(c.T)
    biasT = np.ascontiguousarray(bias[0].transpose(0, 2, 1).astype(bf16))
    id128 = np.eye(128, dtype=bf16)

    in_maps = []
    for cc in range(NCORE):
        sl = slice(cc * NS, (cc + 1) * NS)
        in_maps.append({
            "x": np.ascontiguousarray(x[:, sl, :].reshape(R, C)),
            "cT": cT,
            "bias_t": np.ascontiguousarray(biasT[:, :, sl]),
            "adaln_wT": adaln_wT, "adaln_b4": adaln_b4,
            "qkv_wT": qkv_wT, "qkv_b_bc": qkv_b_bc,
            "qscale_bc": qscale_bc, "kscale_bc": kscale_bc,
            "proj_wT": proj_wT, "proj_b_bc": proj_b_bc,
            "w1T": w1T, "w3T": w3T, "w2T": w2T, "w2_b_bc": w2_b_bc,
            "id128": id128,
        })

    return in_maps


def kernel(**inputs):
    in_maps = prepare_in_maps(inputs)
    nc = _get_nc()
    res = run_bass_kernel_spmd(nc, in_maps, core_ids=list(range(NCORE)))
    _CACHE["last_res"] = res
    out = np.empty((B, N, C), np.float32)
    for cc in range(NCORE):
        out[:, cc * NS:(cc + 1) * NS, :] = res.results[cc]["out"].reshape(B, NS, C)
    return out


# revision 38
# speedup vs baseline: 1.1239x; 1.0061x over previous
"""DiT block kernel for 8 Trainium2 NeuronCores (Bass/Tile).

Sharding: each core owns a 256-wide query slice of the sequence (all batches,
all heads).
 - LN1/modulate/QKV/rmsnorm computed on own rows; K^T (bf16) and V (fp8,
   pre-interleaved with a ones-column per head) are gathered with ONE fused
   AllGather so every core holds full K/V.
 - Attention bias is pre-transposed on host to [H, m, n] and sliced per core
   along n, so every bias element is read exactly once across the machine.
   Bias is pre-loaded into PSUM via an identity matmul, the scores matmul
   accumulates on top, ScalarE applies exp (PSUM -> SBUF fp8e5m2).
 - o^T accumulated on PE in fp8 DoubleRow over m-tile pairs, with the ones
   column so the softmax denominator rides along as psum row 64; the divide
   is folded into the o^T evacuation via a tiny broadcast matmul.
 - proj/MLP are row-local; outputs concatenated on host.

Engine balance: per-head rmsnorm scaling runs on the Pool engine; LN1 stats
overlap the adaLN matmuls; proj/w2 weights prefetch under attention/MLP.
"""

import contextlib

import numpy as np
import ml_dtypes

import concourse.bacc as bacc
import concourse.tile as tile
from concourse import mybir
from concourse.bass_utils import run_bass_kernel_spmd

bf16 = ml_dtypes.bfloat16
F32 = mybir.dt.float32
BF16 = mybir.dt.bfloat16
F8 = mybir.dt.float8e4
F8E5 = mybir.dt.float8e5
AF = mybir.ActivationFunctionType
AL = mybir.AluOpType

B, N, C = 4, 2048, 768
H, D = 12, 64
FFN = 2048
NCORE = 8
NS = N // NCORE          # 256 queries per core
R = B * NS               # 1024 rows per core
RT = R // 128            # 8 row tiles
KT = C // 128            # 6 contraction tiles over C
FT = FFN // 128          # 16 FFN row tiles
EPS_LN, EPS_RMS = 1e-6, 1e-8

VW = H * (D + 1)         # 780: v row width with per-head ones column
VP = 784                 # padded v row stride (fp8 DoubleRow ldweights needs
                         # 16B-aligned tile strides)
KV_K = C * R             # elems (= bytes, fp8) of k^T shard block
KV_TOT = KV_K + R * VP   # fused shard bytes: k^T | v'


def _bc(ap, parts=128):
    """partition-stride-0 broadcast AP (DRAM source)."""
    import dataclasses
    return dataclasses.replace(ap, ap=[[0, parts]] + list(ap.ap))


def _ln_mod(nc, pool, src_ap, sc_bc, sh_bc, dst_bf, eps_tile):
    """dst = LN(src) * sc + sh   (sc already includes the +1)."""
    stats = pool.tile([128, 2, 6], F32, tag="ln_stats", name="ln_stats")
    nc.vector.bn_stats(out=stats[:, 0, :], in_=src_ap[:, 0:384])
    nc.vector.bn_stats(out=stats[:, 1, :], in_=src_ap[:, 384:768])
    mv = pool.tile([128, 2], F32, tag="ln_mv", name="ln_mv")
    nc.vector.bn_aggr(out=mv, in_=stats)
    rstd = pool.tile([128, 1], F32, tag="ln_rstd", name="ln_rstd")
    nc.scalar.activation(out=rstd, in_=mv[:, 1:2], func=AF.Sqrt, bias=eps_tile)
    nc.vector.reciprocal(out=rstd, in_=rstd)
    t1 = pool.tile([128, C], F32, tag="ln_t1", name="ln_t1")
    nc.vector.tensor_scalar(out=t1, in0=src_ap, scalar1=mv[:, 0:1], scalar2=rstd,
                            op0=AL.subtract, op1=AL.mult)
    nc.vector.tensor_tensor(out=t1, in0=t1, in1=sc_bc, op=AL.mult)
    nc.vector.tensor_tensor(out=dst_bf, in0=t1, in1=sh_bc, op=AL.add)


def build(collective=True, repeat=1, ablate=()):
    ablate = frozenset(ablate)
    nc = bacc.Bacc("TRN2", target_bir_lowering=False, debug=False,
                   num_devices=NCORE)

    x_in = nc.dram_tensor("x", [R, C], F32, kind="ExternalInput")
    cT_in = nc.dram_tensor("cT", [C, B], F32, kind="ExternalInput")
    bias_in = nc.dram_tensor("bias_t", [H, N, NS], BF16, kind="ExternalInput")
    adw_in = nc.dram_tensor("adaln_wT", [C, 6 * C], BF16, kind="ExternalInput")
    adb_in = nc.dram_tensor("adaln_b4", [B, 6 * C], F32, kind="ExternalInput")
    qkvw_in = nc.dram_tensor("qkv_wT", [C, 3 * C], BF16, kind="ExternalInput")
    qkvb_in = nc.dram_tensor("qkv_b_bc", [128, 3 * C], F32, kind="ExternalInput")
    qsc_in = nc.dram_tensor("qscale_bc", [128, C], BF16, kind="ExternalInput")
    ksc_in = nc.dram_tensor("kscale_bc", [128, C], BF16, kind="ExternalInput")
    pw_in = nc.dram_tensor("proj_wT", [C, C], BF16, kind="ExternalInput")
    pb_in = nc.dram_tensor("proj_b_bc", [128, C], F32, kind="ExternalInput")
    w1_in = nc.dram_tensor("w1T", [C, FFN], BF16, kind="ExternalInput")
    w3_in = nc.dram_tensor("w3T", [C, FFN], BF16, kind="ExternalInput")
    w2_in = nc.dram_tensor("w2T", [FFN, C], BF16, kind="ExternalInput")
    w2b_in = nc.dram_tensor("w2_b_bc", [128, C], F32, kind="ExternalInput")
    id_in = nc.dram_tensor("id128", [128, 128], BF16, kind="ExternalInput")
    out_t = nc.dram_tensor("out", [R, C], F32, kind="ExternalOutput")

    with tile.TileContext(nc, num_cores=NCORE) as tc, contextlib.ExitStack() as top:
        consts = top.enter_context(tc.tile_pool(name="consts", bufs=1))
        dram = top.enter_context(tc.tile_pool(name="dram", bufs=1, space="DRAM"))
        keep = top.enter_context(tc.tile_pool(name="keep", bufs=1))

        eps_ln = consts.tile([128, 1], F32)
        nc.vector.memset(eps_ln, EPS_LN)
        id_sb = consts.tile([128, 128], BF16)
        nc.sync.dma_start(out=id_sb, in_=id_in[:, :])
        ones_sb = consts.tile([128, 128], BF16)
        nc.vector.memset(ones_sb, 1.0)

        for _rep in range(repeat):
            with contextlib.ExitStack() as ctx:
                qT_sb = keep.tile([128, KT, R], F8)       # packed q^T (fp8)
                oT_sb = keep.tile([128, KT, R], BF16)     # packed normalized o^T
                h2T_sb = keep.tile([128, KT, R], BF16)    # packed LN2-mod x2^T
                mod_dram = dram.tile([B, 6 * C], F32)
                x1_sb = keep.tile([128, RT, C], F32)      # resident input rows
                x2_sb = keep.tile([128, RT, C], F32)      # post-attn residual
                st_sb = keep.tile([128, RT, 2, 6], F32)   # LN1 bn stats
                mv_sb = keep.tile([128, RT, 2], F32)      # LN1 mean/var
                rs_sb = keep.tile([128, RT, 1], F32)      # LN1 rstd

                # ===== P0: adaLN modulation, overlapped with LN1 stats ======
                with tc.tile_pool(name="p0", bufs=1) as p0, \
                     tc.tile_pool(name="p0c", bufs=3) as p0c, \
                     tc.tile_pool(name="p0ps", bufs=2, space="PSUM") as p0ps:
                    cT_sb = p0.tile([128, KT, B], F32)
                    nc.sync.dma_start(
                        out=cT_sb, in_=cT_in.rearrange("(t p) b -> p t b", p=128))
                    scT = p0.tile([128, KT, B], BF16)
                    nc.scalar.activation(out=scT, in_=cT_sb, func=AF.Silu)

                    # LN1 stats for all row tiles: x DMA on gpsimd queue, stats
                    # on DVE — both run under the adaLN matmuls below.
                    for rt in range(RT):
                        nc.gpsimd.dma_start(
                            out=x1_sb[:, rt, :],
                            in_=x_in[rt * 128:(rt + 1) * 128, :])
                        nc.vector.bn_stats(out=st_sb[:, rt, 0, :],
                                           in_=x1_sb[:, rt, 0:384])
                        nc.vector.bn_stats(out=st_sb[:, rt, 1, :],
                                           in_=x1_sb[:, rt, 384:768])
                        nc.vector.bn_aggr(out=mv_sb[:, rt], in_=st_sb[:, rt])
                        nc.scalar.activation(out=rs_sb[:, rt], in_=mv_sb[:, rt, 1:2],
                                             func=AF.Sqrt, bias=eps_ln)
                        nc.vector.reciprocal(out=rs_sb[:, rt], in_=rs_sb[:, rt])

                    adwg = adw_in.rearrange("(t p) j -> p t j", p=128)
                    adb_sb = p0.tile([B, 6 * C], F32)
                    nc.sync.dma_start(out=adb_sb, in_=adb_in[:, :])
                    mod_sb = p0.tile([B, 6 * C], F32)
                    for big in range(3):
                        bsl = slice(big * 1536, (big + 1) * 1536)
                        adw_t = p0c.tile([128, KT, 1536], BF16, tag="adw_t",
                                         name="adw_t")
                        nc.sync.dma_start(out=adw_t, in_=adwg[:, :, bsl])
                        for sub in range(3):
                            c0 = big * 1536 + sub * 512
                            sl = slice(c0, c0 + 512)
                            psM = p0ps.tile([B, 512], F32, tag="psM", name="psM")
                            for kt in range(KT):
                                nc.tensor.matmul(psM, lhsT=scT[:, kt, :],
                                                 rhs=adw_t[:, kt,
                                                           sub * 512:(sub + 1) * 512],
                                                 start=(kt == 0), stop=(kt == KT - 1))
                            nc.vector.tensor_tensor(out=mod_sb[:, sl], in0=psM,
                                                    in1=adb_sb[:, sl], op=AL.add)
                        nc.gpsimd.dma_start(out=mod_dram[:, bsl], in_=mod_sb[:, bsl])

                # ===== P1+P2: LN1 modulate, QKV (k,v first), rmsnorm =====
                # qkv weight columns are host-permuted to [k | v | q] so the K/V
                # side finishes first and the AllGather overlaps Q-side compute.
                # two half-shards (batches 0-1 | 2-3): the first AllGather
                # fires mid-way through the k/v loop and overlaps the rest of
                # P2; attention on b0/b1 overlaps the second gather.
                RH = R // 2
                KV_H = KT * 128 * RH
                TOT_H = KV_H + RH * VP
                addr = "Shared" if collective else "Local"
                kv_shard = [dram.tile([1, TOT_H], F8, tag=f"kvs{h}",
                                      name=f"kvs{h}") for h in range(2)]
                kv_all = [dram.tile([NCORE, TOT_H], F8, addr_space=addr,
                                    tag=f"kva{h}", name=f"kva{h}")
                          for h in range(2)]
                kv_k_view = [s[0, 0:KV_H].rearrange("(t p n) -> p t n",
                                                    p=128, t=KT)
                             for s in kv_shard]
                kv_v_view = [s[0, KV_H:].rearrange("(r c) -> r c", c=VP)
                             for s in kv_shard]

                def emit_gather(h):
                    if "ag" in ablate:
                        return
                    if collective:
                        nc.gpsimd.collective_compute(
                            "AllGather", AL.bypass,
                            replica_groups=[list(range(NCORE))],
                            ins=[kv_shard[h].opt()], outs=[kv_all[h].opt()],
                        )
                    else:
                        for cc in range(2):
                            nc.scalar.dma_start(out=kv_all[h][cc:cc + 1, :],
                                                in_=kv_shard[h][:, :])
                with tc.tile_pool(name="bc1", bufs=1) as bc1, \
                     tc.tile_pool(name="p2", bufs=1) as p2, \
                     tc.tile_pool(name="p2w", bufs=2) as p2w, \
                     tc.tile_pool(name="p2ps", bufs=4, space="PSUM") as p2ps:
                    msa_sc, msa_sh = [], []
                    for b in range(B):
                        # host folded the +1 into adaln_b4's scale segments
                        sc = bc1.tile([128, C], F32, tag=f"sc1_{b}", name=f"sc1_{b}")
                        nc.sync.dma_start(out=sc, in_=_bc(mod_dram[b, C:2 * C]))
                        sh = bc1.tile([128, C], F32, tag=f"sh1_{b}", name=f"sh1_{b}")
                        nc.sync.dma_start(out=sh, in_=_bc(mod_dram[b, 0:C]))
                        msa_sc.append(sc)
                        msa_sh.append(sh)

                    qkvw_sb = p2.tile([128, KT, 3 * C], BF16)
                    nc.scalar.dma_start(
                        out=qkvw_sb, in_=qkvw_in.rearrange("(t p) j -> p t j", p=128))
                    qkvb_sb = p2.tile([128, 3 * C], F32)
                    nc.scalar.dma_start(out=qkvb_sb, in_=qkvb_in[:, :])
                    qsc_sb = p2.tile([128, C], BF16)
                    nc.scalar.dma_start(out=qsc_sb, in_=qsc_in[:, :])
                    ksc_sb = p2.tile([128, C], BF16)
                    nc.scalar.dma_start(out=ksc_sb, in_=ksc_in[:, :])

                    h1T_sb = p2.tile([128, KT, R], BF16)
                    for rt in range(RT):
                        t1 = p2w.tile([128, C], F32, tag="m1_t1", name="m1_t1")
                        nc.vector.tensor_scalar(
                            out=t1, in0=x1_sb[:, rt, :], scalar1=mv_sb[:, rt, 0:1],
                            scalar2=rs_sb[:, rt], op0=AL.subtract, op1=AL.mult)
                        nc.vector.tensor_tensor(out=t1, in0=t1,
                                                in1=msa_sc[rt // 2], op=AL.mult)
                        h1_t = p2w.tile([128, C], BF16, tag="h1_t", name="h1_t")
                        nc.vector.tensor_tensor(out=h1_t, in0=t1,
                                                in1=msa_sh[rt // 2], op=AL.add)
                        nc.sync.dma_start_transpose(
                            out=h1T_sb[:, :, rt * 128:(rt + 1) * 128], in_=h1_t)

                    def qkv_mm(rt, c0, cw):
                        psQ = p2ps.tile([128, 512], F32, tag="psQ", name="psQ")
                        for kt in range(KT):
                            nc.tensor.matmul(
                                psQ[:, 0:cw],
                                lhsT=h1T_sb[:, kt, rt * 128:(rt + 1) * 128],
                                rhs=qkvw_sb[:, kt, c0:c0 + cw],
                                start=(kt == 0), stop=(kt == KT - 1))
                        return psQ

                    def pool_headmul(dst, ss):
                        for h in range(H):
                            hs = slice(h * D, (h + 1) * D)
                            nc.gpsimd.tensor_scalar(
                                out=dst[:, hs], in0=dst[:, hs],
                                scalar1=ss[:, h:h + 1], scalar2=None, op0=AL.mult)

                    def rms_apply(t, scale_sb, dst, tagp):
                        """t: [128, 768] bf16 -> dst normalized bf16."""
                        sq = p2w.tile([128, C], BF16, tag=f"sq{tagp}", name="sq")
                        nc.vector.tensor_tensor(out=sq, in0=t, in1=t, op=AL.mult)
                        ss = p2w.tile([128, H], F32, tag=f"ss{tagp}", name="ss")
                        nc.vector.tensor_reduce(
                            out=ss, in_=sq.rearrange("p (h d) -> p h d", d=D),
                            axis=mybir.AxisListType.X, op=AL.add)
                        nc.scalar.activation(out=ss, in_=ss, func=AF.Sqrt,
                                             scale=1.0 / D)
                        nc.vector.tensor_scalar_add(out=ss, in0=ss, scalar1=EPS_RMS)
                        nc.vector.reciprocal(out=ss, in_=ss)
                        nc.vector.tensor_tensor(out=dst, in0=t, in1=scale_sb,
                                                op=AL.mult)
                        pool_headmul(dst, ss)

                    # ---- k,v side ----
                    for rt in range(RT):
                        rsl = slice(rt * 128, (rt + 1) * 128)
                        kv_t = p2w.tile([128, C], BF16, tag="kv_t", name="kv_t")
                        v8_t = p2w.tile([128, VP], F8, tag="v8_t", name="v8_t")
                        v8h = v8_t[:, 0:VW].rearrange("p (h e) -> p h e", e=D + 1)
                        nc.vector.memset(v8h[:, :, D:D + 1], 1.0)
                        nc.vector.memset(v8_t[:, VW:VP], 0.0)
                        psK = qkv_mm(rt, 0, 512)
                        nc.vector.tensor_tensor(
                            out=kv_t[:, 0:512], in0=psK,
                            in1=qkvb_sb[:, 0:512], op=AL.add)
                        psV = qkv_mm(rt, 1024, 512)
                        with nc.allow_low_precision(reason="fp8 v path validated"):
                            nc.vector.tensor_tensor(
                                out=v8h[:, 4:12, 0:D],
                                in0=psV.rearrange("p (h d) -> p h d", d=D),
                                in1=qkvb_sb[:, 1024:1536].rearrange(
                                    "p (h d) -> p h d", d=D),
                                op=AL.add)
                        # middle chunk straddles k|v: split the evacuation
                        psM2 = qkv_mm(rt, 512, 512)
                        nc.vector.tensor_tensor(
                            out=kv_t[:, 512:768], in0=psM2[:, 0:256],
                            in1=qkvb_sb[:, 512:768], op=AL.add)
                        with nc.allow_low_precision(reason="v in fp8, validated"):
                            nc.vector.tensor_tensor(
                                out=v8h[:, 0:4, 0:D],
                                in0=psM2[:, 256:512].rearrange(
                                    "p (h d) -> p h d", d=D),
                                in1=qkvb_sb[:, 768:1024].rearrange(
                                    "p (h d) -> p h d", d=D),
                                op=AL.add)
                        kn_t = p2w.tile([128, C], BF16, tag="kn_t", name="kn_t")
                        rms_apply(kv_t[:, 0:C], ksc_sb, kn_t, "k")
                        ktr = p2w.tile([128, KT, 128], BF16, tag="ktr", name="ktr")
                        nc.sync.dma_start_transpose(out=ktr, in_=kn_t)
                        k8 = p2w.tile([128, KT, 128], F8, tag="k8", name="k8")
                        with nc.allow_low_precision(reason="fp8 k validated"):
                            nc.vector.tensor_copy(out=k8, in_=ktr)
                        hf, lsl = rt // 4, slice((rt % 4) * 128, (rt % 4 + 1) * 128)
                        nc.sync.dma_start(out=kv_k_view[hf][:, :, lsl], in_=k8)
                        nc.sync.dma_start(out=kv_v_view[hf][lsl, :], in_=v8_t)
                        if rt == 3:
                            emit_gather(0)
                    emit_gather(1)

                    # ---- q side (overlaps the gather) ----
                    for rt in range(RT):
                        q_t = p2w.tile([128, C], BF16, tag="q_t", name="q_t")
                        for c0, cw in ((1536, 512), (2048, 256)):
                            psQ = qkv_mm(rt, c0, cw)
                            nc.vector.tensor_tensor(
                                out=q_t[:, c0 - 1536:c0 - 1536 + cw],
                                in0=psQ[:, 0:cw],
                                in1=qkvb_sb[:, c0:c0 + cw], op=AL.add)
                        qn_t = p2w.tile([128, C], BF16, tag="qn_t", name="qn_t")
                        rms_apply(q_t, qsc_sb, qn_t, "q")
                        qtr = p2w.tile([128, KT, 128], BF16, tag="qtr", name="qtr")
                        nc.sync.dma_start_transpose(out=qtr, in_=qn_t)
                        with nc.allow_low_precision(reason="fp8 q validated"):
                            nc.vector.tensor_copy(
                                out=qT_sb[:, :, rt * 128:(rt + 1) * 128], in_=qtr)

                # ===== P4 prefetch: proj weights stream in under attention ====
                p4pre = ctx.enter_context(tc.tile_pool(name="p4pre", bufs=1))
                pw_sb = p4pre.tile([128, KT, C], BF16)
                nc.scalar.dma_start(
                    out=pw_sb, in_=pw_in.rearrange("(t p) j -> p t j", p=128))
                pb_sb = p4pre.tile([128, C], F32)
                nc.scalar.dma_start(out=pb_sb, in_=pb_in[:, :])

                # ===== P3: attention =====
                if "attn" in ablate:
                    nc.vector.memset(oT_sb, 0.0)
                else:
                  with tc.tile_pool(name="p3v", bufs=1) as p3v, \
                       tc.tile_pool(name="p3b", bufs=3) as p3b, \
                       tc.tile_pool(name="p3k", bufs=4) as p3k, \
                       tc.tile_pool(name="p3a", bufs=4) as p3a, \
                       tc.tile_pool(name="p3r", bufs=2) as p3r, \
                       tc.tile_pool(name="psS", bufs=3, space="PSUM") as psSp, \
                       tc.tile_pool(name="psO", bufs=2, space="PSUM") as psOp:
                    # persistent v' tiles fp8 [128, 64, 784]; ones columns
                    # arrive pre-interleaved from the gather.
                    v_sb = p3v.tile([128, 64, VP], F8)

                    def load_v_tiles(b):
                        for i in range(16):
                            cp, half = i // 2, i % 2
                            r0 = (b % 2) * NS + half * 128
                            src = kv_all[b // 2][cp, KV_H:].rearrange(
                                "(r c) -> r c", c=VP)
                            nc.gpsimd.dma_start(
                                out=v_sb[:, b * 16 + i, :],
                                in_=src[r0:r0 + 128, :])

                    def bias_load(g):
                        bias_g = p3b.tile([128, 2, 16, NS], BF16, tag="bias_g",
                                          name="bias_g")
                        nc.sync.dma_start(
                            out=bias_g,
                            in_=bias_in[2 * g:2 * g + 2].rearrange(
                                "h (i p) n -> p h i n", p=128))
                        return bias_g

                    # two bias groups prefetch on the sync queue during the
                    # gather; v tiles follow once the gather lands.
                    bias_ring = [bias_load(0), bias_load(1), bias_load(2)]
                    load_v_tiles(0)
                    kge = [a[:, 0:KV_H].rearrange("c (r n) -> c r n", n=RH)
                           for a in kv_all]
                    for g in range(KT):
                        bias_g = bias_ring[g % 3]
                        for b in range(B):
                            kT2 = p3k.tile([128, NCORE, NS], F8, tag="kT2",
                                           name="kT2")
                            nc.sync.dma_start(
                                out=kT2,
                                in_=kge[b // 2][:, g * 128:(g + 1) * 128,
                                                (b % 2) * NS:
                                                (b % 2 + 1) * NS].rearrange(
                                            "c p n -> p c n"))
                            kT2f = kT2.rearrange("p c n -> p (c n)")
                            if g == 0 and b < B - 1:
                                load_v_tiles(b + 1)
                            if g < KT - 3 and b == B - 1:
                                bias_ring[g % 3] = bias_load(g + 3)
                            for hh in range(2):
                                h = 2 * g + hh
                                pb = hh * 64
                                psO = psOp.tile([128, 2 * NS], F32, tag="psO",
                                                name="psO")
                                for j4 in range(4):
                                    psS = psSp.tile([128, 1024], F32, tag="psS",
                                                    name="psS")
                                    attnT = p3a.tile([128, 1024], F8E5,
                                                     tag="attnT", name="attnT")
                                    for t2 in range(2):
                                        nc.tensor.matmul(
                                            psS[:, t2 * 512:(t2 + 1) * 512],
                                            lhsT=id_sb,
                                            rhs=bias_g[:, hh,
                                                       4 * j4 + 2 * t2:
                                                       4 * j4 + 2 * t2 + 2, :],
                                            start=True, stop=False,
                                            skip_group_check=True)
                                    for t in range(4):
                                        i = 4 * j4 + t
                                        tsl = slice(t * 256, (t + 1) * 256)
                                        nc.tensor.matmul(
                                            psS[:, tsl],
                                            lhsT=kT2f[pb:pb + 64,
                                                      i * 128:(i + 1) * 128],
                                            rhs=qT_sb[pb:pb + 64, g,
                                                      b * NS:(b + 1) * NS],
                                            start=False, stop=True)
                                    with nc.allow_low_precision(
                                            reason="fp8 attn validated 1.5e-3"):
                                        nc.scalar.activation(out=attnT, in_=psS,
                                                             func=AF.Exp)
                                    for pr in range(2):
                                        j = 2 * j4 + pr
                                        nc.tensor.matmul(
                                            psO[0:D + 1, 0:NS],
                                            lhsT=v_sb[:, b * 16 + 2 * j:
                                                      b * 16 + 2 * j + 2,
                                                      h * (D + 1):
                                                      (h + 1) * (D + 1)],
                                            rhs=attnT[:, 2 * pr * 256:
                                                      (2 * pr + 2) * 256].rearrange(
                                                "p (two n) -> p two n", two=2),
                                            start=(j == 0), stop=(j == 7),
                                            perf_mode=mybir.MatmulPerfMode.DoubleRow)
                                rs = p3r.tile([128, NS], BF16, tag="rs", name="rs")
                                with nc.allow_low_precision(
                                        reason="bf16 softmax denom reciprocal, "
                                               "validated 1.2e-3 end-to-end"):
                                    nc.vector.reciprocal(out=rs[64:65, :],
                                                         in_=psO[D:D + 1, 0:NS])
                                nc.tensor.matmul(psO[:, NS:2 * NS],
                                                 lhsT=ones_sb[64:65, :],
                                                 rhs=rs[64:65, :], start=True,
                                                 stop=True, skip_group_check=True)
                                rb = p3r.tile([128, NS], BF16, tag="rb", name="rb")
                                nc.vector.tensor_copy(out=rb[0:D, :],
                                                      in_=psO[0:D, NS:2 * NS])
                                nc.vector.tensor_tensor(
                                    out=oT_sb[pb:pb + 64, g, b * NS:(b + 1) * NS],
                                    in0=psO[0:D, 0:NS], in1=rb[0:D, :], op=AL.mult)

                # ===== P4: proj + residual =====
                with tc.tile_pool(name="p4w", bufs=3) as p4w, \
                     tc.tile_pool(name="p4ps", bufs=3, space="PSUM") as p4ps, \
                     tc.tile_pool(name="bc2", bufs=1) as bc2:
                    g1_bc = []
                    mlp_sc, mlp_sh = [], []
                    for b in range(B):
                        g1 = bc2.tile([128, C], F32, tag=f"g1_{b}", name=f"g1_{b}")
                        nc.sync.dma_start(out=g1, in_=_bc(mod_dram[b, 2 * C:3 * C]))
                        g1_bc.append(g1)
                        sc = bc2.tile([128, C], F32, tag=f"sc2_{b}", name=f"sc2_{b}")
                        nc.sync.dma_start(out=sc, in_=_bc(mod_dram[b, 4 * C:5 * C]))
                        sh = bc2.tile([128, C], F32, tag=f"sh2_{b}", name=f"sh2_{b}")
                        nc.sync.dma_start(out=sh, in_=_bc(mod_dram[b, 3 * C:4 * C]))
                        mlp_sc.append(sc)
                        mlp_sh.append(sh)
                    for rt in range(RT):
                        t1 = p4w.tile([128, C], F32, tag="pj_t1", name="pj_t1")
                        for c0, cw in ((0, 512), (512, 256)):
                            psP = p4ps.tile([128, 512], F32, tag="psP", name="psP")
                            for kt in range(KT):
                                nc.tensor.matmul(
                                    psP[:, 0:cw],
                                    lhsT=oT_sb[:, kt, rt * 128:(rt + 1) * 128],
                                    rhs=pw_sb[:, kt, c0:c0 + cw],
                                    start=(kt == 0), stop=(kt == KT - 1))
                            nc.vector.tensor_tensor(out=t1[:, c0:c0 + cw],
                                                    in0=psP[:, 0:cw],
                                                    in1=pb_sb[:, c0:c0 + cw],
                                                    op=AL.add)
                        t2 = p4w.tile([128, C], F32, tag="pj_t2", name="pj_t2")
                        nc.vector.tensor_tensor(out=t2, in0=t1, in1=g1_bc[rt // 2],
                                                op=AL.mult)
                        nc.vector.tensor_tensor(out=x2_sb[:, rt, :], in0=t2,
                                                in1=x1_sb[:, rt, :], op=AL.add)
                        # LN2 + modulate pipelined right behind each proj tile
                        h2_t = p4w.tile([128, C], BF16, tag="h2_t", name="h2_t")
                        _ln_mod(nc, p4w, x2_sb[:, rt, :], mlp_sc[rt // 2],
                                mlp_sh[rt // 2], h2_t, eps_ln)
                        nc.sync.dma_start_transpose(
                            out=h2T_sb[:, :, rt * 128:(rt + 1) * 128], in_=h2_t)

                # ===== P5: SwiGLU MLP =====
                if "mlp" in ablate:
                    for rt in range(RT):
                        nc.sync.dma_start(out=out_t[rt * 128:(rt + 1) * 128, :],
                                          in_=x2_sb[:, rt, :])
                else:
                  with tc.tile_pool(name="p5z", bufs=1) as p5z:
                    zT_sb = p5z.tile([128, FT, R], BF16)
                    # w2 prefetches on the scalar queue under the z loop
                    w2_sb = p5z.tile([128, FT, C], BF16)
                    nc.scalar.dma_start(
                        out=w2_sb, in_=w2_in.rearrange("(t p) j -> p t j", p=128))
                    w2b_sb = p5z.tile([128, C], F32)
                    nc.scalar.dma_start(out=w2b_sb, in_=w2b_in[:, :])
                    with tc.tile_pool(name="p5w", bufs=3) as p5w, \
                         tc.tile_pool(name="p5ps", bufs=2, space="PSUM") as p5ps:
                        w1g = w1_in.rearrange("(t p) j -> p t j", p=128)
                        w3g = w3_in.rearrange("(t p) j -> p t j", p=128)
                        for fc in range(FT // 2):
                            fsl = slice(fc * 256, (fc + 1) * 256)
                            w1_t = p5w.tile([128, KT, 256], BF16, tag="w1_t",
                                            name="w1_t")
                            nc.sync.dma_start(out=w1_t, in_=w1g[:, :, fsl])
                            w3_t = p5w.tile([128, KT, 256], BF16, tag="w3_t",
                                            name="w3_t")
                            nc.sync.dma_start(out=w3_t, in_=w3g[:, :, fsl])
                            for sub in range(2):
                                ft = fc * 2 + sub
                                ssl = slice(sub * 128, (sub + 1) * 128)
                                for nch in range(2):
                                    nsl = slice(nch * 512, (nch + 1) * 512)
                                    psU = p5ps.tile([128, 512], F32, tag="psU",
                                                    name="psU")
                                    psG = p5ps.tile([128, 512], F32, tag="psG",
                                                    name="psG")
                                    for kt in range(KT):
                                        nc.tensor.matmul(psU, lhsT=w1_t[:, kt, ssl],
                                                         rhs=h2T_sb[:, kt, nsl],
                                                         start=(kt == 0),
                                                         stop=(kt == KT - 1))
                                    for kt in range(KT):
                                        nc.tensor.matmul(psG, lhsT=w3_t[:, kt, ssl],
                                                         rhs=h2T_sb[:, kt, nsl],
                                                         start=(kt == 0),
                                                         stop=(kt == KT - 1))
                                    us = p5w.tile([128, 512], BF16, tag="us",
                                                  name="us")
                                    nc.scalar.activation(out=us, in_=psU,
                                                         func=AF.Silu)
                                    nc.vector.tensor_tensor(out=zT_sb[:, ft, nsl],
                                                            in0=us, in1=psG,
                                                            op=AL.mult)

                    # ---- second half: z @ w2 + gate + residual ----
                    with tc.tile_pool(name="bc4", bufs=1) as bc4, \
                         tc.tile_pool(name="p6w", bufs=3) as p6w, \
                         tc.tile_pool(name="p6ps", bufs=2, space="PSUM") as p6ps:
                        g2_bc = []
                        for b in range(B):
                            g2 = bc4.tile([128, C], F32, tag=f"g2_{b}",
                                          name=f"g2_{b}")
                            nc.sync.dma_start(out=g2,
                                              in_=_bc(mod_dram[b, 5 * C:6 * C]))
                            g2_bc.append(g2)
                        for rt in range(RT):
                            t1 = p6w.tile([128, C], F32, tag="o2_t1", name="o2_t1")
                            for c0, cw in ((0, 512), (512, 256)):
                                psP = p6ps.tile([128, 512], F32, tag="psO2",
                                                name="psO2")
                                for kt in range(FT):
                                    nc.tensor.matmul(
                                        psP[:, 0:cw],
                                        lhsT=zT_sb[:, kt, rt * 128:(rt + 1) * 128],
                                        rhs=w2_sb[:, kt, c0:c0 + cw],
                                        start=(kt == 0), stop=(kt == FT - 1))
                                nc.vector.tensor_tensor(out=t1[:, c0:c0 + cw],
                                                        in0=psP[:, 0:cw],
                                                        in1=w2b_sb[:, c0:c0 + cw],
                                                        op=AL.add)
                            t2 = p6w.tile([128, C], F32, tag="o2_t2", name="o2_t2")
                            nc.vector.tensor_tensor(out=t2, in0=t1,
                                                    in1=g2_bc[rt // 2], op=AL.mult)
                            y_t = p6w.tile([128, C], F32, tag="y_t", name="y_t")
                            nc.vector.tensor_tensor(out=y_t, in0=t2,
                                                    in1=x2_sb[:, rt, :], op=AL.add)
                            nc.sync.dma_start(out=out_t[rt * 128:(rt + 1) * 128, :],
                                              in_=y_t)

    nc.compile()
    return nc


_CACHE = {}


def _get_nc():
    if "nc" not in _CACHE:
        _CACHE["nc"] = build()
    return _CACHE["nc"]


def prepare_in_maps(inputs):
    inputs = {k: np.asarray(v) for k, v in inputs.items()}
    x = inputs["x"].astype(np.float32)
    c = inputs["c"].astype(np.float32)
    bias = inputs["bias"].astype(np.float32)
    q_scale = inputs["q_scale"].astype(np.float32)
    k_scale = inputs["k_scale"].astype(np.float32)

    qkv_w_f = inputs["qkv_w"].astype(np.float32)          # rows [q; k; v]
    qkv_w_kvq = np.concatenate(
        [qkv_w_f[C:2 * C], qkv_w_f[2 * C:], qkv_w_f[:C]], axis=0)
    qkv_wT = np.ascontiguousarray(qkv_w_kvq.T.astype(bf16))
    proj_wT = np.ascontiguousarray(inputs["proj_w"].astype(np.float32).T.astype(bf16))
    w1T = np.ascontiguousarray(inputs["w1"].astype(np.float32).T.astype(bf16))
    w3T = np.ascontiguousarray(inputs["w3"].astype(np.float32).T.astype(bf16))
    w2T = np.ascontiguousarray(inputs["w2_w"].astype(np.float32).T.astype(bf16))
    adaln_wT = np.ascontiguousarray(
        inputs["adaln_w"].astype(np.float32).T.astype(bf16))
    adaln_b4 = np.broadcast_to(
        inputs["adaln_b"].astype(np.float32), (B, 6 * C)).copy()
    adaln_b4[:, C:2 * C] += 1.0      # fold modulate's (1 + scale)
    adaln_b4[:, 4 * C:5 * C] += 1.0
    qkv_b_f = inputs["qkv_b"].astype(np.float32)
    qkv_b_kvq = np.concatenate([qkv_b_f[C:2 * C], qkv_b_f[2 * C:], qkv_b_f[:C]])
    qkv_b_bc = np.ascontiguousarray(np.broadcast_to(qkv_b_kvq, (128, 3 * C)))
    proj_b_bc = np.ascontiguousarray(
        np.broadcast_to(inputs["proj_b"].astype(np.float32), (128, C)))
    w2_b_bc = np.ascontiguousarray(
        np.broadcast_to(inputs["w2_b"].astype(np.float32), (128, C)))
    qscale_bc = np.ascontiguousarray(np.broadcast_to(
        np.tile(q_scale * D ** -0.5, H).astype(bf16), (128, C)))
    kscale_bc = np.ascontiguousarray(np.broadcast_to(
        np.tile(k_scale, H).astype(bf16), (128, C)))
    cT = np.ascontiguousarray(c.T)
    biasT = np.ascontiguousarray(bias[0].transpose(0, 2, 1).astype(bf16))
    id128 = np.eye(128, dtype=bf16)

    in_maps = []
    for cc in range(NCORE):
        sl = slice(cc * NS, (cc + 1) * NS)
        in_maps.append({
            "x": np.ascontiguousarray(x[:, sl, :].reshape(R, C)),
            "cT": cT,
            "bias_t": np.ascontiguousarray(biasT[:, :, sl]),
            "adaln_wT": adaln_wT, "adaln_b4": adaln_b4,
            "qkv_wT": qkv_wT, "qkv_b_bc": qkv_b_bc,
            "qscale_bc": qscale_bc, "kscale_bc": kscale_bc,
            "proj_wT": proj_wT, "proj_b_bc": proj_b_bc,
            "w1T": w1T, "w3T": w3T, "w2T": w2T, "w2_b_bc": w2_b_bc,
            "id128": id128,
        })

    return in_maps


def kernel(**inputs):
    in_maps = prepare_in_maps(inputs)
    nc = _get_nc()
    res = run_bass_kernel_spmd(nc, in_maps, core_ids=list(range(NCORE)))
    _CACHE["last_res"] = res
    out = np.empty((B, N, C), np.float32)
    for cc in range(NCORE):
        out[:, cc * NS:(cc + 1) * NS, :] = res.results[cc]["out"].reshape(B, NS, C)
    return out
